# revision 9
# baseline (speedup 1.0000x reference)
"""Trainium2 Bass kernel for nn_CausalPredictor.

Math (per image y = x[b], all f32):
    zd   = dic @ Wz_w.T + Wz_b                          [K, C]
    att  = softmax((y @ Wy_w.T + Wy_b) @ zd.T * s, k)   [L, K]
    z    = (att * prior) @ dic                          [L, D]
    ly   = y @ cs_w[:, :D].T                            [L, C]
    lz   = z @ cs_w[:, D:].T + cs_b                     [L, C]
    out[i*L+j, c] = ly[i, c] + lz[j, c]                 [L*L, C]

Rewritten to avoid materializing z and to keep every big matmul contraction
on the partition dim:
    zdts = (zd.T + Wz_b) * s            [C, K]
    M    = Wy_w.T @ zdts                [D, K]
    b2   = Wy_b @ zdts                  [K]
    ep   = exp(M.T @ y.T + b2)          [K, L]   (logits, transposed)
    G    = diag(prior) @ (dic @ csz.T)  [K, C];  Gb = [G | ones]  [K, C+1]
    nd   = ep_slice.T @ Gb              [128j, C+1]  (num | denom)
    lz   = nd[:, :C] / nd[:, C:] + cs_b
    out block = [lyT; ones].T @ [I_C tiled; lz_flat]   (K=C+1 matmul)

Sharding: 8 cores = 4 images x 2 halves of the i dim. No collectives.
Each core gets its full image (for lz over all j) plus its i-half rows
(for ly), computes a [512, 1024, 21] slab of the output.
"""

import sys

for _p in ("/opt/trn_rl_repo", "/root/.axon_site/_ro/trn_rl_repo"):
    if _p not in sys.path:
        sys.path.append(_p)

import numpy as np

import concourse.bass as bass
from concourse import bacc
import concourse.mybir as mybir
import concourse.tile as tile
from concourse.masks import make_identity
from contextlib import ExitStack

B, L, D, K, C = 4, 1024, 1024, 20, 21
SCALE = 1.0 / float(np.sqrt(np.float32(C)))
F32 = mybir.dt.float32
HALF_L = L // 2          # 512 rows of i per core
N_IC = HALF_L // 128     # 4 i-chunks of 128 per core
N_DC = D // 128          # 8 chunks along the contraction dim
JC = 512                 # j columns per rhs_big fill (one j-half of 1024)
RHS_W = JC * C           # 10752 free elements in rhs_big
Q_N = RHS_W // 512       # 21 matmuls of N=512 per (jh, ic)
OUT_Q = 7                # q's per staged output tile
OUT_W = OUT_Q * 512      # 3584 f32 per partition per staged tile


def _build_program():
    nc = bacc.Bacc(
        "TRN2",
        target_bir_lowering=False,
        debug=False,
        enable_asserts=False,
        num_devices=8,
    )
    y_img = nc.dram_tensor("y_img", [L, D], F32, kind="ExternalInput").ap()
    y_half = nc.dram_tensor("y_half", [HALF_L, D], F32, kind="ExternalInput").ap()
    dic = nc.dram_tensor("dic", [K, D], F32, kind="ExternalInput").ap()
    prior = nc.dram_tensor("prior", [K], F32, kind="ExternalInput").ap()
    wy_w = nc.dram_tensor("Wy_w", [C, D], F32, kind="ExternalInput").ap()
    wy_b = nc.dram_tensor("Wy_b", [C], F32, kind="ExternalInput").ap()
    wz_w = nc.dram_tensor("Wz_w", [C, D], F32, kind="ExternalInput").ap()
    wz_b = nc.dram_tensor("Wz_b", [C], F32, kind="ExternalInput").ap()
    cs_w = nc.dram_tensor("cs_w", [C, 2 * D], F32, kind="ExternalInput").ap()
    cs_b = nc.dram_tensor("cs_b", [C], F32, kind="ExternalInput").ap()
    out = nc.dram_tensor("out_loc", [HALF_L, L * C], F32, kind="ExternalOutput").ap()

    with tile.TileContext(nc) as tc:
        _emit(tc, out, y_img, y_half, dic, prior, wy_w, wy_b, wz_w, wz_b, cs_w, cs_b)
    nc.compile()
    return nc


def _bcast_ap(ap, parts):
    """Partition-broadcast a 1-D DRAM AP across `parts` partitions (DMA only)."""
    return bass.AP(tensor=ap.tensor, offset=ap.offset, ap=[[0, parts]] + list(ap.ap))


def _emit(tc, out, y_img, y_half, dic, prior, wy_w, wy_b, wz_w, wz_b, cs_w, cs_b):
    nc = tc.nc
    ctx = ExitStack()
    with ctx:
        consts = ctx.enter_context(tc.tile_pool(name="consts", bufs=1))
        ypool = ctx.enter_context(tc.tile_pool(name="ypool", bufs=3))
        outpool = ctx.enter_context(tc.tile_pool(name="outpool", bufs=3))

        # ---- constant loads -------------------------------------------------
        ident = consts.tile([128, 128], F32, name="ident")
        make_identity(nc, ident)

        dic_sb = consts.tile([K, D], F32, name="dic_sb")
        nc.sync.dma_start(out=dic_sb, in_=dic)
        wy_sb = consts.tile([C, D], F32, name="wy_sb")
        nc.sync.dma_start(out=wy_sb, in_=wy_w)
        wz_sb = consts.tile([C, D], F32, name="wz_sb")
        nc.sync.dma_start(out=wz_sb, in_=wz_w)
        cs_sb = consts.tile([C, 2 * D], F32, name="cs_sb")
        nc.sync.dma_start(out=cs_sb, in_=cs_w)

        prior_col = consts.tile([K, 1], F32, name="prior_col")
        nc.sync.dma_start(out=prior_col, in_=prior.unsqueeze(1))
        wyb_col = consts.tile([C, 1], F32, name="wyb_col")
        nc.sync.dma_start(out=wyb_col, in_=wy_b.unsqueeze(1))
        wzb_col = consts.tile([C, 1], F32, name="wzb_col")
        nc.sync.dma_start(out=wzb_col, in_=wz_b.unsqueeze(1))
        csb_rep = consts.tile([128, C], F32, name="csb_rep")
        nc.sync.dma_start(out=csb_rep, in_=_bcast_ap(cs_b, 128))

        # ---- prologue: transposed weights + tiny matmuls --------------------
        pro_ctx = ExitStack()
        pro_ps = pro_ctx.enter_context(tc.tile_pool(name="pro_ps", bufs=2, space="PSUM"))

        dicT = consts.tile([128, N_DC, K], F32, name="dicT")
        wzT = consts.tile([128, N_DC, C], F32, name="wzT")
        csyT = consts.tile([128, N_DC, C], F32, name="csyT")
        cszT = consts.tile([128, N_DC, C], F32, name="cszT")
        for dc in range(N_DC):
            sl = slice(dc * 128, (dc + 1) * 128)
            pt = pro_ps.tile([128, K], F32, name="pt", tag="pt")
            nc.tensor.transpose(pt, dic_sb[:, sl], ident[:K, :K])
            nc.scalar.copy(dicT[:, dc, :], pt)
            pw = pro_ps.tile([128, C], F32, name="pw", tag="pw")
            nc.tensor.transpose(pw, wz_sb[:, sl], ident[:C, :C])
            nc.scalar.copy(wzT[:, dc, :], pw)
            py = pro_ps.tile([128, C], F32, name="py", tag="pw")
            nc.tensor.transpose(py, cs_sb[:, sl], ident[:C, :C])
            nc.scalar.copy(csyT[:, dc, :], py)
            pz = pro_ps.tile([128, C], F32, name="pz", tag="pw")
            nc.tensor.transpose(pz, cs_sb[:, D + dc * 128 : D + (dc + 1) * 128], ident[:C, :C])
            nc.scalar.copy(cszT[:, dc, :], pz)

        # zdts = (Wz @ dic.T + Wz_b) * scale      [C, K]
        ps_zd = pro_ps.tile([C, K], F32, name="ps_zd", tag="small")
        for dc in range(N_DC):
            nc.tensor.matmul(ps_zd, wzT[:, dc, :], dicT[:, dc, :],
                             start=(dc == 0), stop=(dc == N_DC - 1))
        zdts = consts.tile([C, K], F32, name="zdts")
        nc.vector.tensor_scalar(zdts, ps_zd, wzb_col, SCALE,
                                op0=mybir.AluOpType.add, op1=mybir.AluOpType.mult)

        # M = Wy_w.T @ zdts   [D, K] in 8 chunks of [128, K]
        m_sb = consts.tile([128, N_DC, K], F32, name="m_sb")
        for dc in range(N_DC):
            ps_m = pro_ps.tile([128, K], F32, name="ps_m", tag="pt")
            nc.tensor.matmul(ps_m, wy_sb[:, dc * 128 : (dc + 1) * 128], zdts)
            nc.scalar.copy(m_sb[:, dc, :], ps_m)

        # b2 = Wy_b @ zdts -> column [K, 1] (exp bias)
        ps_b2 = pro_ps.tile([1, K], F32, name="ps_b2", tag="small")
        nc.tensor.matmul(ps_b2, wyb_col, zdts)
        b2_row = consts.tile([1, K], F32, name="b2_row")
        nc.scalar.copy(b2_row, ps_b2)
        ps_b2t = pro_ps.tile([K, 1], F32, name="ps_b2t", tag="small")
        nc.tensor.transpose(ps_b2t, b2_row, ident[:1, :1])
        ebias = consts.tile([K, 1], F32, name="ebias")
        nc.scalar.copy(ebias, ps_b2t)

        # Gb = [diag(prior) @ dic @ csz.T | ones]   [K, C+1]
        ps_g = pro_ps.tile([K, C], F32, name="ps_g", tag="small")
        for dc in range(N_DC):
            nc.tensor.matmul(ps_g, dicT[:, dc, :], cszT[:, dc, :],
                             start=(dc == 0), stop=(dc == N_DC - 1))
        gb = consts.tile([K, C + 1], F32, name="gb")
        nc.vector.tensor_scalar_mul(gb[:, 0:C], ps_g, prior_col)
        nc.vector.memset(gb[:, C : C + 1], 1.0)

        # rhs_big rows 0..C-1: I_C tiled 512x along j (constant)
        rhs_big = consts.tile([C + 1, RHS_W], F32, name="rhs_big")
        nc.gpsimd.memset(rhs_big[0:C, :], 0.0)
        nc.gpsimd.affine_select(
            out=rhs_big[0:C, :].rearrange("p (j c) -> p j c", c=C),
            in_=rhs_big[0:C, :].rearrange("p (j c) -> p j c", c=C),
            compare_op=mybir.AluOpType.not_equal,
            fill=1.0,
            base=0,
            pattern=[[0, JC], [1, C]],
            channel_multiplier=-1,
        )

        pro_ctx.close()

        # ---- phase 1: per-image transposes and small matmuls ----------------
        p1_ctx = ExitStack()
        tr_ps = p1_ctx.enter_context(tc.tile_pool(name="tr_ps", bufs=4, space="PSUM"))
        u_ps = p1_ctx.enter_context(tc.tile_pool(name="u_ps", bufs=1, space="PSUM"))

        # y.T for the full image: 8 tiles [128 d, 1024 l]
        yT = [consts.tile([128, L], F32, name=f"yT{dc}") for dc in range(N_DC)]
        flip = 0
        for lc in range(L // 128):
            yt = ypool.tile([128, D], F32, name="yt", tag="yt")
            nc.sync.dma_start(out=yt, in_=y_img[lc * 128 : (lc + 1) * 128, :])
            for dc in range(N_DC):
                pt = tr_ps.tile([128, 128], F32, name="ptr", tag="ptr")
                nc.tensor.transpose(pt, yt[:, dc * 128 : (dc + 1) * 128], ident)
                dst = yT[dc][:, lc * 128 : (lc + 1) * 128]
                if flip % 2 == 0:
                    nc.vector.tensor_copy(dst, pt)
                else:
                    nc.scalar.copy(dst, pt)
                flip += 1

        # y_half.T: 8 tiles [128 d, 512 l]
        yhT = [consts.tile([128, HALF_L], F32, name=f"yhT{dc}") for dc in range(N_DC)]
        for lc in range(HALF_L // 128):
            yt = ypool.tile([128, D], F32, name="yht", tag="yt")
            nc.sync.dma_start(out=yt, in_=y_half[lc * 128 : (lc + 1) * 128, :])
            for dc in range(N_DC):
                pt = tr_ps.tile([128, 128], F32, name="ptr2", tag="ptr")
                nc.tensor.transpose(pt, yt[:, dc * 128 : (dc + 1) * 128], ident)
                dst = yhT[dc][:, lc * 128 : (lc + 1) * 128]
                if flip % 2 == 0:
                    nc.vector.tensor_copy(dst, pt)
                else:
                    nc.scalar.copy(dst, pt)
                flip += 1

        # ep = exp(M.T @ y.T + b2)   [K, L]
        ep = consts.tile([K, L], F32, name="ep")
        for ns in range(2):
            ps_u = u_ps.tile([K, 512], F32, name="ps_u", tag="ps_u")
            for dc in range(N_DC):
                nc.tensor.matmul(ps_u, m_sb[:, dc, :],
                                 yT[dc][:, ns * 512 : (ns + 1) * 512],
                                 start=(dc == 0), stop=(dc == N_DC - 1))
            nc.scalar.activation(ep[:, ns * 512 : (ns + 1) * 512], ps_u,
                                 mybir.ActivationFunctionType.Exp, bias=ebias, scale=1.0)

        # ly_lhsT rows 0..C-1 = (y_half @ csy.T).T, row C = ones
        # (compute engines can't start at partition 21, so stage the ones row
        #  at partition 0 and DMA it into place)
        ly_lhsT = consts.tile([C + 1, HALF_L], F32, name="ly_lhsT")
        ones_row = consts.tile([1, HALF_L], F32, name="ones_row")
        nc.vector.memset(ones_row, 1.0)
        nc.sync.dma_start(out=ly_lhsT[C : C + 1, :], in_=ones_row)
        ps_ly = u_ps.tile([C, HALF_L], F32, name="ps_ly", tag="ps_ly")
        for dc in range(N_DC):
            nc.tensor.matmul(ps_ly, csyT[:, dc, :], yhT[dc],
                             start=(dc == 0), stop=(dc == N_DC - 1))
        nc.scalar.copy(ly_lhsT[0:C, :], ps_ly)

        # lz chunks: [128 j, C] x 8
        lz_pool = ctx.enter_context(tc.tile_pool(name="lz_pool", bufs=2))
        lz_sb = consts.tile([128, N_DC, C], F32, name="lz_sb")
        for jc in range(N_DC):
            ps_nd = u_ps.tile([128, C + 1], F32, name="ps_nd", tag="ps_nd")
            nc.tensor.matmul(ps_nd, ep[:, jc * 128 : (jc + 1) * 128], gb)
            recip = lz_pool.tile([128, 1], F32, name="recip", tag="recip")
            nc.vector.reciprocal(recip, ps_nd[:, C : C + 1])
            nc.vector.scalar_tensor_tensor(
                lz_sb[:, jc, :], ps_nd[:, 0:C], recip, csb_rep,
                op0=mybir.AluOpType.mult, op1=mybir.AluOpType.add)

        p1_ctx.close()

        # ---- phase 2: outer-sum matmuls + staged output ---------------------
        out_ps = ctx.enter_context(tc.tile_pool(name="out_ps", bufs=8, space="PSUM"))
        out_v = out  # [HALF_L, L*C]

        for jh in range(2):
            # rhs_big row C <- lz_flat for this j-half (partition -> free flatten)
            for ch in range(4):
                nc.sync.dma_start(
                    out=rhs_big[C : C + 1, ch * 128 * C : (ch + 1) * 128 * C],
                    in_=lz_sb[:, jh * 4 + ch, :],
                )
            for ic in range(N_IC):
                lhs = ly_lhsT[:, ic * 128 : (ic + 1) * 128]
                for qg in range(Q_N // OUT_Q):
                    ob = outpool.tile([128, OUT_W], F32, name="ob", tag="ob")
                    for qq in range(OUT_Q):
                        q = qg * OUT_Q + qq
                        po = out_ps.tile([128, 512], F32, name="po", tag="po")
                        nc.tensor.matmul(po, lhs, rhs_big[:, q * 512 : (q + 1) * 512])
                        dst = ob[:, qq * 512 : (qq + 1) * 512]
                        if q % 2 == 0:
                            nc.vector.tensor_copy(dst, po)
                        else:
                            nc.scalar.copy(dst, po)
                    nc.sync.dma_start(
                        out=out_v[ic * 128 : (ic + 1) * 128,
                                  jh * RHS_W + qg * OUT_W : jh * RHS_W + (qg + 1) * OUT_W],
                        in_=ob,
                    )


_NC_CACHE = None


def _get_nc():
    global _NC_CACHE
    if _NC_CACHE is None:
        _NC_CACHE = _build_program()
    return _NC_CACHE


def make_in_maps(inputs):
    x = np.ascontiguousarray(np.asarray(inputs["x"], dtype=np.float32))
    small = {
        k: np.ascontiguousarray(np.asarray(inputs[k], dtype=np.float32))
        for k in ("dic", "prior", "Wy_w", "Wy_b", "Wz_w", "Wz_b", "cs_w", "cs_b")
    }
    in_maps = []
    for core in range(8):
        b, ihalf = core % B, core // B
        in_maps.append({
            "y_img": x[b],
            "y_half": np.ascontiguousarray(x[b, ihalf * HALF_L : (ihalf + 1) * HALF_L]),
            **small,
        })
    return in_maps


def assemble(results):
    out = np.empty((B, L, L, C), dtype=np.float32)
    for core in range(8):
        b, ihalf = core % B, core // B
        out[b, ihalf * HALF_L : (ihalf + 1) * HALF_L] = (
            results[core]["out_loc"].reshape(HALF_L, L, C)
        )
    return out.reshape(B, L * L, C)


def _install_trace_support():
    """The agent image's antenv lacks axon_hooks, so boot() skipped NTFF hook
    install. Recreate the module and register the ctypes-based hook; also stub
    the S3 artifact upload (no creds in this container)."""
    import types

    if sys.modules.get("antenv.axon_hooks") is None:
        mod = types.ModuleType("antenv.axon_hooks")
        _hook = [None]
        mod.set_axon_ntff_profile_hook = lambda h: _hook.__setitem__(0, h)
        mod.get_axon_ntff_profile_hook = lambda: _hook[0]
        sys.modules["antenv.axon_hooks"] = mod
        import antenv

        antenv.axon_hooks = mod
    import antenv.axon_hooks as ah

    if ah.get_axon_ntff_profile_hook() is None:
        from trn_agent_boot.trn_boot import _ntff_profile_via_ctypes

        ah.set_axon_ntff_profile_hook(
            _ntff_profile_via_ctypes("/opt/axon/libaxon_pjrt.so")
        )
    import concourse.bass_utils as bu

    bu.upload_artifacts = lambda tmpdir: tmpdir


def run(inputs, trace=False, **kw):
    from concourse.bass_utils import run_bass_kernel_spmd

    if trace:
        _install_trace_support()
    nc = _get_nc()
    res = run_bass_kernel_spmd(
        nc, make_in_maps(inputs), core_ids=list(range(8)), trace=trace, **kw
    )
    return assemble(res.results), res


def kernel(**inputs) -> np.ndarray:
    out, _ = run(inputs, trace=False)
    return out


# revision 15
# speedup vs baseline: 1.4542x; 1.4542x over previous
"""Trainium2 Bass kernel for nn_CausalPredictor.

Math (per image y = x[b], all f32):
    zd   = dic @ Wz_w.T + Wz_b                          [K, C]
    att  = softmax((y @ Wy_w.T + Wy_b) @ zd.T * s, k)   [L, K]
    z    = (att * prior) @ dic                          [L, D]
    ly   = y @ cs_w[:, :D].T                            [L, C]
    lz   = z @ cs_w[:, D:].T + cs_b                     [L, C]
    out[i*L+j, c] = ly[i, c] + lz[j, c]                 [L*L, C]

Rewritten to avoid materializing z and to keep every big matmul contraction
on the partition dim:
    zdts = (zd.T + Wz_b) * s            [C, K]
    M    = Wy_w.T @ zdts                [D, K]
    b2   = Wy_b @ zdts                  [K]
    ep   = exp(M.T @ y.T + b2)          [K, L]   (logits, transposed)
    G    = diag(prior) @ (dic @ csz.T)  [K, C];  Gb = [G | ones]  [K, C+1]
    nd   = ep_slice.T @ Gb              [128j, C+1]  (num | denom)
    lz   = nd[:, :C] / nd[:, C:] + cs_b
    out block = [lyT; ones].T @ [I_C tiled; lz_flat]   (K=C+1 matmul)

Sharding: 8 cores = 4 images x 2 halves of the i dim. No collectives.
Each core gets its full image (for lz over all j) plus its i-half rows
(for ly), computes a [512, 1024, 21] slab of the output.
"""

import sys

for _p in ("/opt/trn_rl_repo", "/root/.axon_site/_ro/trn_rl_repo"):
    if _p not in sys.path:
        sys.path.append(_p)

import numpy as np

import concourse.bass as bass
from concourse import bacc
import concourse.mybir as mybir
import concourse.tile as tile
from concourse.masks import make_identity
from contextlib import ExitStack

B, L, D, K, C = 4, 1024, 1024, 20, 21
SCALE = 1.0 / float(np.sqrt(np.float32(C)))
F32 = mybir.dt.float32
BF16 = mybir.dt.bfloat16
HALF_L = L // 2          # 512 rows of i per core
N_IC = HALF_L // 128     # 4 i-chunks of 128 per core
N_DC = D // 128          # 8 chunks along the contraction dim
JC = 512                 # j columns per rhs_big fill (one j-half of 1024)
RHS_W = JC * C           # 10752 free elements in rhs_big
Q_N = RHS_W // 512       # 21 matmuls of N=512 per (jh, ic)
OUT_Q = 7                # q's per staged output tile
OUT_W = OUT_Q * 512      # 3584 f32 per partition per staged tile


def _build_program():
    nc = bacc.Bacc(
        "TRN2",
        target_bir_lowering=False,
        debug=False,
        enable_asserts=False,
        num_devices=8,
    )
    y_img = nc.dram_tensor("y_img", [L, D], F32, kind="ExternalInput").ap()
    y_half = nc.dram_tensor("y_half", [HALF_L, D], F32, kind="ExternalInput").ap()
    dic = nc.dram_tensor("dic", [K, D], F32, kind="ExternalInput").ap()
    prior = nc.dram_tensor("prior", [K], F32, kind="ExternalInput").ap()
    wy_w = nc.dram_tensor("Wy_w", [C, D], F32, kind="ExternalInput").ap()
    wy_b = nc.dram_tensor("Wy_b", [C], F32, kind="ExternalInput").ap()
    wz_w = nc.dram_tensor("Wz_w", [C, D], F32, kind="ExternalInput").ap()
    wz_b = nc.dram_tensor("Wz_b", [C], F32, kind="ExternalInput").ap()
    cs_w = nc.dram_tensor("cs_w", [C, 2 * D], F32, kind="ExternalInput").ap()
    cs_b = nc.dram_tensor("cs_b", [C], F32, kind="ExternalInput").ap()
    out = nc.dram_tensor("out_loc", [HALF_L, L * C], F32, kind="ExternalOutput").ap()

    with tile.TileContext(nc) as tc:
        _emit(tc, out, y_img, y_half, dic, prior, wy_w, wy_b, wz_w, wz_b, cs_w, cs_b)
    nc.compile()
    return nc


def _bcast_ap(ap, parts):
    """Partition-broadcast a 1-D DRAM AP across `parts` partitions (DMA only)."""
    return bass.AP(tensor=ap.tensor, offset=ap.offset, ap=[[0, parts]] + list(ap.ap))


def _emit(tc, out, y_img, y_half, dic, prior, wy_w, wy_b, wz_w, wz_b, cs_w, cs_b):
    nc = tc.nc
    ctx = ExitStack()
    with ctx:
        consts = ctx.enter_context(tc.tile_pool(name="consts", bufs=1))
        ypool = ctx.enter_context(tc.tile_pool(name="ypool", bufs=3))
        outpool = ctx.enter_context(tc.tile_pool(name="outpool", bufs=3))

        # ---- constant loads -------------------------------------------------
        ident = consts.tile([128, 128], F32, name="ident")
        make_identity(nc, ident)

        dic_sb = consts.tile([K, D], F32, name="dic_sb")
        nc.sync.dma_start(out=dic_sb, in_=dic)
        wy_sb = consts.tile([C, D], F32, name="wy_sb")
        nc.sync.dma_start(out=wy_sb, in_=wy_w)
        wz_sb = consts.tile([C, D], F32, name="wz_sb")
        nc.sync.dma_start(out=wz_sb, in_=wz_w)
        cs_sb = consts.tile([C, 2 * D], F32, name="cs_sb")
        nc.sync.dma_start(out=cs_sb, in_=cs_w)

        prior_col = consts.tile([K, 1], F32, name="prior_col")
        nc.sync.dma_start(out=prior_col, in_=prior.unsqueeze(1))
        wyb_col = consts.tile([C, 1], F32, name="wyb_col")
        nc.sync.dma_start(out=wyb_col, in_=wy_b.unsqueeze(1))
        wzb_col = consts.tile([C, 1], F32, name="wzb_col")
        nc.sync.dma_start(out=wzb_col, in_=wz_b.unsqueeze(1))
        csb_rep = consts.tile([128, C], F32, name="csb_rep")
        nc.sync.dma_start(out=csb_rep, in_=_bcast_ap(cs_b, 128))

        # ---- prologue: transposed weights + tiny matmuls --------------------
        pro_ctx = ExitStack()
        pro_ps = pro_ctx.enter_context(tc.tile_pool(name="pro_ps", bufs=2, space="PSUM"))

        dicT = consts.tile([128, N_DC, K], F32, name="dicT")
        wzT = consts.tile([128, N_DC, C], F32, name="wzT")
        csyT = consts.tile([128, N_DC, C], F32, name="csyT")
        cszT = consts.tile([128, N_DC, C], F32, name="cszT")
        for dc in range(N_DC):
            sl = slice(dc * 128, (dc + 1) * 128)
            pt = pro_ps.tile([128, K], F32, name="pt", tag="pt")
            nc.tensor.transpose(pt, dic_sb[:, sl], ident[:K, :K])
            nc.scalar.copy(dicT[:, dc, :], pt)
            pw = pro_ps.tile([128, C], F32, name="pw", tag="pw")
            nc.tensor.transpose(pw, wz_sb[:, sl], ident[:C, :C])
            nc.scalar.copy(wzT[:, dc, :], pw)
            py = pro_ps.tile([128, C], F32, name="py", tag="pw")
            nc.tensor.transpose(py, cs_sb[:, sl], ident[:C, :C])
            nc.scalar.copy(csyT[:, dc, :], py)
            pz = pro_ps.tile([128, C], F32, name="pz", tag="pw")
            nc.tensor.transpose(pz, cs_sb[:, D + dc * 128 : D + (dc + 1) * 128], ident[:C, :C])
            nc.scalar.copy(cszT[:, dc, :], pz)

        # zdts = (Wz @ dic.T + Wz_b) * scale      [C, K]
        ps_zd = pro_ps.tile([C, K], F32, name="ps_zd", tag="small")
        for dc in range(N_DC):
            nc.tensor.matmul(ps_zd, wzT[:, dc, :], dicT[:, dc, :],
                             start=(dc == 0), stop=(dc == N_DC - 1))
        zdts = consts.tile([C, K], F32, name="zdts")
        nc.vector.tensor_scalar(zdts, ps_zd, wzb_col, SCALE,
                                op0=mybir.AluOpType.add, op1=mybir.AluOpType.mult)

        # M = Wy_w.T @ zdts   [D, K] in 8 chunks of [128, K]
        m_sb = consts.tile([128, N_DC, K], F32, name="m_sb")
        for dc in range(N_DC):
            ps_m = pro_ps.tile([128, K], F32, name="ps_m", tag="pt")
            nc.tensor.matmul(ps_m, wy_sb[:, dc * 128 : (dc + 1) * 128], zdts)
            nc.scalar.copy(m_sb[:, dc, :], ps_m)

        # b2 = Wy_b @ zdts -> column [K, 1] (exp bias)
        ps_b2 = pro_ps.tile([1, K], F32, name="ps_b2", tag="small")
        nc.tensor.matmul(ps_b2, wyb_col, zdts)
        b2_row = consts.tile([1, K], F32, name="b2_row")
        nc.scalar.copy(b2_row, ps_b2)
        ps_b2t = pro_ps.tile([K, 1], F32, name="ps_b2t", tag="small")
        nc.tensor.transpose(ps_b2t, b2_row, ident[:1, :1])
        ebias = consts.tile([K, 1], F32, name="ebias")
        nc.scalar.copy(ebias, ps_b2t)

        # Gb = [diag(prior) @ dic @ csz.T | ones]   [K, C+1]
        ps_g = pro_ps.tile([K, C], F32, name="ps_g", tag="small")
        for dc in range(N_DC):
            nc.tensor.matmul(ps_g, dicT[:, dc, :], cszT[:, dc, :],
                             start=(dc == 0), stop=(dc == N_DC - 1))
        gb = consts.tile([K, C + 1], F32, name="gb")
        nc.vector.tensor_scalar_mul(gb[:, 0:C], ps_g, prior_col)
        nc.vector.memset(gb[:, C : C + 1], 1.0)

        # rhs_big: 99 x RHS_W bf16.  Rows 32s..32s+20 hold I_C tiled 512x along
        # j (one copy per bf16 mantissa split s); rows 96..98 hold the three
        # bf16 splits of lz_flat for the current j-half (written per image).
        # The output matmul contracts K=99 in bf16 (1 cyc/row on PE vs 4 for
        # fp32 LOW_HIGH), and hi+mid+lo recovers full fp32 precision since
        # every product is value * {0,1} (exact) accumulated in fp32 PSUM.
        rhs_big = consts.tile([99, RHS_W], BF16, name="rhs_big")
        nc.gpsimd.memset(rhs_big, 0.0)
        nc.gpsimd.affine_select(
            out=rhs_big[0:C, :].rearrange("p (j c) -> p j c", c=C),
            in_=rhs_big[0:C, :].rearrange("p (j c) -> p j c", c=C),
            compare_op=mybir.AluOpType.not_equal,
            fill=1.0,
            base=0,
            pattern=[[0, JC], [1, C]],
            channel_multiplier=-1,
        )
        nc.sync.dma_start(out=rhs_big[32 : 32 + C, :], in_=rhs_big[0:C, :])
        nc.sync.dma_start(out=rhs_big[64 : 64 + C, :], in_=rhs_big[0:C, :])

        pro_ctx.close()

        # ---- phase 1: per-image transposes and small matmuls ----------------
        p1_ctx = ExitStack()
        tr_ps = p1_ctx.enter_context(tc.tile_pool(name="tr_ps", bufs=4, space="PSUM"))
        u_ps = p1_ctx.enter_context(tc.tile_pool(name="u_ps", bufs=1, space="PSUM"))

        # y.T for the full image: 8 tiles [128 d, 1024 l]
        yT = [consts.tile([128, L], F32, name=f"yT{dc}") for dc in range(N_DC)]
        flip = 0
        for lc in range(L // 128):
            yt = ypool.tile([128, D], F32, name="yt", tag="yt")
            nc.sync.dma_start(out=yt, in_=y_img[lc * 128 : (lc + 1) * 128, :])
            for dc in range(N_DC):
                pt = tr_ps.tile([128, 128], F32, name="ptr", tag="ptr")
                nc.tensor.transpose(pt, yt[:, dc * 128 : (dc + 1) * 128], ident)
                dst = yT[dc][:, lc * 128 : (lc + 1) * 128]
                if flip % 2 == 0:
                    nc.vector.tensor_copy(dst, pt)
                else:
                    nc.scalar.copy(dst, pt)
                flip += 1

        # y_half.T: 8 tiles [128 d, 512 l]
        yhT = [consts.tile([128, HALF_L], F32, name=f"yhT{dc}") for dc in range(N_DC)]
        for lc in range(HALF_L // 128):
            yt = ypool.tile([128, D], F32, name="yht", tag="yt")
            nc.sync.dma_start(out=yt, in_=y_half[lc * 128 : (lc + 1) * 128, :])
            for dc in range(N_DC):
                pt = tr_ps.tile([128, 128], F32, name="ptr2", tag="ptr")
                nc.tensor.transpose(pt, yt[:, dc * 128 : (dc + 1) * 128], ident)
                dst = yhT[dc][:, lc * 128 : (lc + 1) * 128]
                if flip % 2 == 0:
                    nc.vector.tensor_copy(dst, pt)
                else:
                    nc.scalar.copy(dst, pt)
                flip += 1

        # ep = exp(M.T @ y.T + b2)   [K, L]
        ep = consts.tile([K, L], F32, name="ep")
        for ns in range(2):
            ps_u = u_ps.tile([K, 512], F32, name="ps_u", tag="ps_u")
            for dc in range(N_DC):
                nc.tensor.matmul(ps_u, m_sb[:, dc, :],
                                 yT[dc][:, ns * 512 : (ns + 1) * 512],
                                 start=(dc == 0), stop=(dc == N_DC - 1))
            nc.scalar.activation(ep[:, ns * 512 : (ns + 1) * 512], ps_u,
                                 mybir.ActivationFunctionType.Exp, bias=ebias, scale=1.0)

        # ly_lhsT: bf16 [99, 512].  Rows 32s..32s+20 = mantissa split s of
        # (y_half @ csy.T).T; rows 96..98 = ones (multiply the lz rows of
        # rhs_big).  All row blocks start at 32-aligned partitions.
        ly_lhsT = consts.tile([99, HALF_L], BF16, name="ly_lhsT")
        nc.vector.memset(ly_lhsT, 0.0)
        nc.vector.memset(ly_lhsT[96:99, :], 1.0)
        ps_ly = u_ps.tile([C, HALF_L], F32, name="ps_ly", tag="ps_ly")
        for dc in range(N_DC):
            nc.tensor.matmul(ps_ly, csyT[:, dc, :], yhT[dc],
                             start=(dc == 0), stop=(dc == N_DC - 1))
        # splits staged at base partition 0 (engine ops can't mix base
        # partitions), then DMA'd into the 32-aligned row blocks
        ly_hi_b = consts.tile([C, HALF_L], BF16, name="ly_hi_b")
        ly_mid_b = consts.tile([C, HALF_L], BF16, name="ly_mid_b")
        ly_lo_b = consts.tile([C, HALF_L], BF16, name="ly_lo_b")
        ly_rem1 = consts.tile([C, HALF_L], F32, name="ly_rem1")
        ly_rem2 = consts.tile([C, HALF_L], F32, name="ly_rem2")
        nc.scalar.copy(ly_hi_b, ps_ly)
        nc.vector.tensor_sub(ly_rem1, ps_ly, ly_hi_b)
        nc.scalar.copy(ly_mid_b, ly_rem1)
        nc.vector.tensor_sub(ly_rem2, ly_rem1, ly_mid_b)
        nc.scalar.copy(ly_lo_b, ly_rem2)
        nc.sync.dma_start(out=ly_lhsT[0:C, :], in_=ly_hi_b)
        nc.sync.dma_start(out=ly_lhsT[32 : 32 + C, :], in_=ly_mid_b)
        nc.sync.dma_start(out=ly_lhsT[64 : 64 + C, :], in_=ly_lo_b)

        # lz chunks: [128 j, C] x 8, then 3-way bf16 mantissa split
        lz_pool = ctx.enter_context(tc.tile_pool(name="lz_pool", bufs=2))
        lz_sb = consts.tile([128, N_DC, C], F32, name="lz_sb")
        for jc in range(N_DC):
            ps_nd = u_ps.tile([128, C + 1], F32, name="ps_nd", tag="ps_nd")
            nc.tensor.matmul(ps_nd, ep[:, jc * 128 : (jc + 1) * 128], gb)
            recip = lz_pool.tile([128, 1], F32, name="recip", tag="recip")
            nc.vector.reciprocal(recip, ps_nd[:, C : C + 1])
            nc.vector.scalar_tensor_tensor(
                lz_sb[:, jc, :], ps_nd[:, 0:C], recip, csb_rep,
                op0=mybir.AluOpType.mult, op1=mybir.AluOpType.add)
        lz_hi = consts.tile([128, N_DC, C], BF16, name="lz_hi")
        lz_mid = consts.tile([128, N_DC, C], BF16, name="lz_mid")
        lz_lo = consts.tile([128, N_DC, C], BF16, name="lz_lo")
        lz_rem1 = consts.tile([128, N_DC, C], F32, name="lz_rem1")
        lz_rem2 = consts.tile([128, N_DC, C], F32, name="lz_rem2")
        nc.vector.tensor_copy(lz_hi, lz_sb)
        nc.vector.tensor_sub(lz_rem1, lz_sb, lz_hi)
        nc.vector.tensor_copy(lz_mid, lz_rem1)
        nc.vector.tensor_sub(lz_rem2, lz_rem1, lz_mid)
        nc.vector.tensor_copy(lz_lo, lz_rem2)
        lz_splits = [lz_hi, lz_mid, lz_lo]

        p1_ctx.close()

        # ---- phase 2: outer-sum matmuls + staged output ---------------------
        out_ps = ctx.enter_context(tc.tile_pool(name="out_ps", bufs=8, space="PSUM"))
        out_v = out  # [HALF_L, L*C]

        for jh in range(2):
            # rhs_big rows 96..98 <- bf16 splits of lz_flat for this j-half
            # (partition -> free flatten via SBUF->SBUF DMA)
            for s in range(3):
                for ch in range(4):
                    nc.sync.dma_start(
                        out=rhs_big[96 + s : 97 + s, ch * 128 * C : (ch + 1) * 128 * C],
                        in_=lz_splits[s][:, jh * 4 + ch, :],
                    )
            for ic in range(N_IC):
                lhs = ly_lhsT[:, ic * 128 : (ic + 1) * 128]
                for qg in range(Q_N // OUT_Q):
                    ob = outpool.tile([128, OUT_W], F32, name="ob", tag="ob")
                    for qq in range(OUT_Q):
                        q = qg * OUT_Q + qq
                        po = out_ps.tile([128, 512], F32, name="po", tag="po")
                        nc.tensor.matmul(po, lhs, rhs_big[:, q * 512 : (q + 1) * 512])
                        dst = ob[:, qq * 512 : (qq + 1) * 512]
                        if q % 2 == 0:
                            nc.vector.tensor_copy(dst, po)
                        else:
                            nc.scalar.copy(dst, po)
                    nc.sync.dma_start(
                        out=out_v[ic * 128 : (ic + 1) * 128,
                                  jh * RHS_W + qg * OUT_W : jh * RHS_W + (qg + 1) * OUT_W],
                        in_=ob,
                    )


_NC_CACHE = None


def _get_nc():
    global _NC_CACHE
    if _NC_CACHE is None:
        _NC_CACHE = _build_program()
    return _NC_CACHE


def make_in_maps(inputs):
    x = np.ascontiguousarray(np.asarray(inputs["x"], dtype=np.float32))
    small = {
        k: np.ascontiguousarray(np.asarray(inputs[k], dtype=np.float32))
        for k in ("dic", "prior", "Wy_w", "Wy_b", "Wz_w", "Wz_b", "cs_w", "cs_b")
    }
    in_maps = []
    for core in range(8):
        b, ihalf = core % B, core // B
        in_maps.append({
            "y_img": x[b],
            "y_half": np.ascontiguousarray(x[b, ihalf * HALF_L : (ihalf + 1) * HALF_L]),
            **small,
        })
    return in_maps


def assemble(results):
    out = np.empty((B, L, L, C), dtype=np.float32)
    for core in range(8):
        b, ihalf = core % B, core // B
        out[b, ihalf * HALF_L : (ihalf + 1) * HALF_L] = (
            results[core]["out_loc"].reshape(HALF_L, L, C)
        )
    return out.reshape(B, L * L, C)


def _install_trace_support():
    """The agent image's antenv lacks axon_hooks, so boot() skipped NTFF hook
    install. Recreate the module and register the ctypes-based hook; also stub
    the S3 artifact upload (no creds in this container)."""
    import types

    if sys.modules.get("antenv.axon_hooks") is None:
        mod = types.ModuleType("antenv.axon_hooks")
        _hook = [None]
        mod.set_axon_ntff_profile_hook = lambda h: _hook.__setitem__(0, h)
        mod.get_axon_ntff_profile_hook = lambda: _hook[0]
        sys.modules["antenv.axon_hooks"] = mod
        import antenv

        antenv.axon_hooks = mod
    import antenv.axon_hooks as ah

    if ah.get_axon_ntff_profile_hook() is None:
        from trn_agent_boot.trn_boot import _ntff_profile_via_ctypes

        ah.set_axon_ntff_profile_hook(
            _ntff_profile_via_ctypes("/opt/axon/libaxon_pjrt.so")
        )
    import concourse.bass_utils as bu

    bu.upload_artifacts = lambda tmpdir: tmpdir


def run(inputs, trace=False, **kw):
    from concourse.bass_utils import run_bass_kernel_spmd

    if trace:
        _install_trace_support()
    nc = _get_nc()
    res = run_bass_kernel_spmd(
        nc, make_in_maps(inputs), core_ids=list(range(8)), trace=trace, **kw
    )
    return assemble(res.results), res


def kernel(**inputs) -> np.ndarray:
    out, _ = run(inputs, trace=False)
    return out


# revision 17
# speedup vs baseline: 1.5245x; 1.0483x over previous
"""Trainium2 Bass kernel for nn_CausalPredictor.

Math (per image y = x[b], all f32):
    zd   = dic @ Wz_w.T + Wz_b                          [K, C]
    att  = softmax((y @ Wy_w.T + Wy_b) @ zd.T * s, k)   [L, K]
    z    = (att * prior) @ dic                          [L, D]
    ly   = y @ cs_w[:, :D].T                            [L, C]
    lz   = z @ cs_w[:, D:].T + cs_b                     [L, C]
    out[i*L+j, c] = ly[i, c] + lz[j, c]                 [L*L, C]

Device-side rewrite (keeps every big matmul contraction on the partition dim
and never materializes z):
    zdts = (zd.T + Wz_b) * s            [C, K]
    M    = Wy_w.T @ zdts                [D, K]
    b2   = Wy_b @ zdts                  [K]
    ep   = exp(M.T @ y.T + b2)          [K, L]   (logits, transposed)
    G    = diag(prior) @ (dic @ csz.T)  [K, C];  Gb = [G | ones]  [K, C+1]
    nd   = ep_slice.T @ Gb              [128j, C+1]  (num | denom)
    lz   = nd[:, :C] / nd[:, C:] + cs_b
    out block = lhsT.T @ rhs            (K=99 bf16 matmul per 128-row block)

The outer sum runs on the PE in bf16 at 1 cycle/row (fp32 matmul is 4):
both ly and lz are split into hi+mid+lo bf16 mantissa parts, the rhs holds
three copies of a tiled identity (rows 32s..32s+20) plus the three lz_flat
splits (rows 96..98), and the lhsT holds the three lyT splits plus ones rows.
Every product is value * {0,1} (exact in bf16), accumulated in fp32 PSUM, so
the result is fp32-exact to ~2^-24.

Sharding: 8 cores = 4 images x 2 halves of the i dim, no collectives.  Each
core gets its full image (for lz over all j) plus its i-half rows (for ly)
and computes a [512, 1024, 21] slab.  Work is pipelined over j-halves so the
second half of the attention path hides under the first half's output DMA.
"""

import sys

for _p in ("/opt/trn_rl_repo", "/root/.axon_site/_ro/trn_rl_repo"):
    if _p not in sys.path:
        sys.path.append(_p)

import numpy as np

import concourse.bass as bass
from concourse import bacc
import concourse.mybir as mybir
import concourse.tile as tile
from concourse.masks import make_identity
from contextlib import ExitStack

B, L, D, K, C = 4, 1024, 1024, 20, 21
SCALE = 1.0 / float(np.sqrt(np.float32(C)))
F32 = mybir.dt.float32
BF16 = mybir.dt.bfloat16
HALF_L = L // 2          # 512 rows of i per core
N_IC = HALF_L // 128     # 4 i-chunks of 128 per core
N_DC = D // 128          # 8 chunks along the contraction dim
JC = 512                 # j columns covered by one rhs tile (one j-half)
RHS_W = JC * C           # 10752 free elements per rhs tile
Q_N = RHS_W // 512       # 21 matmuls of N=512 per (jh, ic)
OUT_Q = 7                # q's per staged output tile
OUT_W = OUT_Q * 512      # 3584 f32 per partition per staged tile


def _build_program():
    nc = bacc.Bacc(
        "TRN2",
        target_bir_lowering=False,
        debug=False,
        enable_asserts=False,
        num_devices=8,
    )
    y_img = nc.dram_tensor("y_img", [L, D], F32, kind="ExternalInput").ap()
    y_half = nc.dram_tensor("y_half", [HALF_L, D], F32, kind="ExternalInput").ap()
    dic = nc.dram_tensor("dic", [K, D], F32, kind="ExternalInput").ap()
    prior = nc.dram_tensor("prior", [K], F32, kind="ExternalInput").ap()
    wy_w = nc.dram_tensor("Wy_w", [C, D], F32, kind="ExternalInput").ap()
    wy_b = nc.dram_tensor("Wy_b", [C], F32, kind="ExternalInput").ap()
    wz_w = nc.dram_tensor("Wz_w", [C, D], F32, kind="ExternalInput").ap()
    wz_b = nc.dram_tensor("Wz_b", [C], F32, kind="ExternalInput").ap()
    cs_w = nc.dram_tensor("cs_w", [C, 2 * D], F32, kind="ExternalInput").ap()
    cs_b = nc.dram_tensor("cs_b", [C], F32, kind="ExternalInput").ap()
    out = nc.dram_tensor("out_loc", [HALF_L, L * C], F32, kind="ExternalOutput").ap()

    with tile.TileContext(nc) as tc:
        _emit(tc, out, y_img, y_half, dic, prior, wy_w, wy_b, wz_w, wz_b, cs_w, cs_b)
    nc.compile()
    return nc


def _bcast_ap(ap, parts):
    """Partition-broadcast a 1-D DRAM AP across `parts` partitions (DMA only)."""
    return bass.AP(tensor=ap.tensor, offset=ap.offset, ap=[[0, parts]] + list(ap.ap))


def _emit(tc, out, y_img, y_half, dic, prior, wy_w, wy_b, wz_w, wz_b, cs_w, cs_b):
    nc = tc.nc
    ctx = ExitStack()
    with ctx:
        consts = ctx.enter_context(tc.tile_pool(name="consts", bufs=1))
        ypool = ctx.enter_context(tc.tile_pool(name="ypool", bufs=3))
        outpool = ctx.enter_context(tc.tile_pool(name="outpool", bufs=3))
        lz_pool = ctx.enter_context(tc.tile_pool(name="lz_pool", bufs=2))
        # One flat PSUM layout, no pool releases (releases serialize phases):
        # tr 2 banks + sm 2 banks + out 4 banks = 8.
        tr_ps = ctx.enter_context(tc.tile_pool(name="tr_ps", bufs=2, space="PSUM"))
        sm_ps = ctx.enter_context(tc.tile_pool(name="sm_ps", bufs=2, space="PSUM"))
        out_ps = ctx.enter_context(tc.tile_pool(name="out_ps", bufs=4, space="PSUM"))

        def tr_tile():
            return tr_ps.tile([128, 128], F32, name="tr", tag="tr")

        def sm_tile(p, f):
            return sm_ps.tile([p, f], F32, name="sm", tag="sm")

        # ---- constant loads -------------------------------------------------
        ident = consts.tile([128, 128], F32, name="ident")
        make_identity(nc, ident)

        dic_sb = consts.tile([K, D], F32, name="dic_sb")
        nc.sync.dma_start(out=dic_sb, in_=dic)
        wy_sb = consts.tile([C, D], F32, name="wy_sb")
        nc.sync.dma_start(out=wy_sb, in_=wy_w)
        wz_sb = consts.tile([C, D], F32, name="wz_sb")
        nc.sync.dma_start(out=wz_sb, in_=wz_w)
        cs_sb = consts.tile([C, 2 * D], F32, name="cs_sb")
        nc.sync.dma_start(out=cs_sb, in_=cs_w)

        prior_col = consts.tile([K, 1], F32, name="prior_col")
        nc.sync.dma_start(out=prior_col, in_=prior.unsqueeze(1))
        wyb_col = consts.tile([C, 1], F32, name="wyb_col")
        nc.sync.dma_start(out=wyb_col, in_=wy_b.unsqueeze(1))
        wzb_col = consts.tile([C, 1], F32, name="wzb_col")
        nc.sync.dma_start(out=wzb_col, in_=wz_b.unsqueeze(1))
        csb_rep = consts.tile([128, C], F32, name="csb_rep")
        nc.sync.dma_start(out=csb_rep, in_=_bcast_ap(cs_b, 128))

        # ---- prologue: transposed weights + tiny matmuls --------------------
        dicT = consts.tile([128, N_DC, K], F32, name="dicT")
        wzT = consts.tile([128, N_DC, C], F32, name="wzT")
        csyT = consts.tile([128, N_DC, C], F32, name="csyT")
        cszT = consts.tile([128, N_DC, C], F32, name="cszT")
        for dc in range(N_DC):
            sl = slice(dc * 128, (dc + 1) * 128)
            for src, dst, kk in (
                (dic_sb[:, sl], dicT[:, dc, :], K),
                (wz_sb[:, sl], wzT[:, dc, :], C),
                (cs_sb[:, sl], csyT[:, dc, :], C),
                (cs_sb[:, D + dc * 128 : D + (dc + 1) * 128], cszT[:, dc, :], C),
            ):
                pt = tr_tile()
                nc.tensor.transpose(pt[:, :kk], src, ident[:kk, :kk])
                nc.scalar.copy(dst, pt[:, :kk])

        # zdts = (Wz @ dic.T + Wz_b) * scale      [C, K]
        ps_zd = sm_tile(C, K)
        for dc in range(N_DC):
            nc.tensor.matmul(ps_zd, wzT[:, dc, :], dicT[:, dc, :],
                             start=(dc == 0), stop=(dc == N_DC - 1))
        zdts = consts.tile([C, K], F32, name="zdts")
        nc.vector.tensor_scalar(zdts, ps_zd, wzb_col, SCALE,
                                op0=mybir.AluOpType.add, op1=mybir.AluOpType.mult)

        # M = Wy_w.T @ zdts   [D, K] in 8 chunks of [128, K]
        m_sb = consts.tile([128, N_DC, K], F32, name="m_sb")
        for dc in range(N_DC):
            ps_m = sm_tile(128, K)
            nc.tensor.matmul(ps_m, wy_sb[:, dc * 128 : (dc + 1) * 128], zdts)
            nc.scalar.copy(m_sb[:, dc, :], ps_m)

        # b2 = Wy_b @ zdts -> column [K, 1] (exp bias)
        ps_b2 = sm_tile(1, K)
        nc.tensor.matmul(ps_b2, wyb_col, zdts)
        b2_row = consts.tile([1, K], F32, name="b2_row")
        nc.scalar.copy(b2_row, ps_b2)
        ps_b2t = sm_tile(K, 1)
        nc.tensor.transpose(ps_b2t, b2_row, ident[:1, :1])
        ebias = consts.tile([K, 1], F32, name="ebias")
        nc.scalar.copy(ebias, ps_b2t)

        # Gb = [diag(prior) @ dic @ csz.T | ones]   [K, C+1]
        ps_g = sm_tile(K, C)
        for dc in range(N_DC):
            nc.tensor.matmul(ps_g, dicT[:, dc, :], cszT[:, dc, :],
                             start=(dc == 0), stop=(dc == N_DC - 1))
        gb = consts.tile([K, C + 1], F32, name="gb")
        nc.vector.tensor_scalar_mul(gb[:, 0:C], ps_g, prior_col)
        nc.vector.memset(gb[:, C : C + 1], 1.0)

        # rhs tiles (one per j-half): rows 32s..32s+20 = tiled I_C per split,
        # rows 96..98 = bf16 splits of this j-half's lz_flat.
        rhs = [consts.tile([99, RHS_W], BF16, name=f"rhs{jh}") for jh in range(2)]
        nc.gpsimd.memset(rhs[0], 0.0)
        nc.gpsimd.memset(rhs[1], 0.0)
        nc.gpsimd.affine_select(
            out=rhs[0][0:C, :].rearrange("p (j c) -> p j c", c=C),
            in_=rhs[0][0:C, :].rearrange("p (j c) -> p j c", c=C),
            compare_op=mybir.AluOpType.not_equal,
            fill=1.0,
            base=0,
            pattern=[[0, JC], [1, C]],
            channel_multiplier=-1,
        )
        for dst, row in ((0, 32), (0, 64), (1, 0), (1, 32), (1, 64)):
            nc.scalar.dma_start(out=rhs[dst][row : row + C, :], in_=rhs[0][0:C, :])

        # ---- ly path: y_half -> lyT -> bf16 splits in ly_lhsT ---------------
        yhT = [consts.tile([128, HALF_L], F32, name=f"yhT{dc}") for dc in range(N_DC)]
        for lc in range(HALF_L // 128):
            yt = ypool.tile([128, D], F32, name="yht", tag="yt")
            nc.sync.dma_start(out=yt, in_=y_half[lc * 128 : (lc + 1) * 128, :])
            for dc in range(N_DC):
                pt = tr_tile()
                nc.tensor.transpose(pt, yt[:, dc * 128 : (dc + 1) * 128], ident)
                dst = yhT[dc][:, lc * 128 : (lc + 1) * 128]
                if (lc * N_DC + dc) % 2 == 0:
                    nc.vector.tensor_copy(dst, pt)
                else:
                    nc.scalar.copy(dst, pt)

        ly_lhsT = consts.tile([99, HALF_L], BF16, name="ly_lhsT")
        nc.vector.memset(ly_lhsT, 0.0)
        nc.vector.memset(ly_lhsT[96:99, :], 1.0)
        ps_ly = sm_tile(C, HALF_L)
        for dc in range(N_DC):
            nc.tensor.matmul(ps_ly, csyT[:, dc, :], yhT[dc],
                             start=(dc == 0), stop=(dc == N_DC - 1))
        ly_hi_b = consts.tile([C, HALF_L], BF16, name="ly_hi_b")
        ly_mid_b = consts.tile([C, HALF_L], BF16, name="ly_mid_b")
        ly_lo_b = consts.tile([C, HALF_L], BF16, name="ly_lo_b")
        ly_rem1 = consts.tile([C, HALF_L], F32, name="ly_rem1")
        ly_rem2 = consts.tile([C, HALF_L], F32, name="ly_rem2")
        nc.scalar.copy(ly_hi_b, ps_ly)
        nc.vector.tensor_sub(ly_rem1, ps_ly, ly_hi_b)
        nc.scalar.copy(ly_mid_b, ly_rem1)
        nc.vector.tensor_sub(ly_rem2, ly_rem1, ly_mid_b)
        nc.scalar.copy(ly_lo_b, ly_rem2)
        nc.scalar.dma_start(out=ly_lhsT[0:C, :], in_=ly_hi_b)
        nc.scalar.dma_start(out=ly_lhsT[32 : 32 + C, :], in_=ly_mid_b)
        nc.scalar.dma_start(out=ly_lhsT[64 : 64 + C, :], in_=ly_lo_b)

        # ---- per-j-half attention path + outer-sum --------------------------
        yT = [consts.tile([128, L], F32, name=f"yT{dc}") for dc in range(N_DC)]
        ep = consts.tile([K, L], F32, name="ep")
        lz_sb = consts.tile([128, N_DC, C], F32, name="lz_sb")
        lz_hi = consts.tile([128, N_DC, C], BF16, name="lz_hi")
        lz_mid = consts.tile([128, N_DC, C], BF16, name="lz_mid")
        lz_lo = consts.tile([128, N_DC, C], BF16, name="lz_lo")
        lz_rem1 = consts.tile([128, N_DC, C], F32, name="lz_rem1")
        lz_rem2 = consts.tile([128, N_DC, C], F32, name="lz_rem2")
        lz_splits = [lz_hi, lz_mid, lz_lo]
        flip = 0

        for jh in range(2):
            jsl = slice(jh * 512, (jh + 1) * 512)
            # transpose this j-half's y rows
            for lc in range(jh * 4, (jh + 1) * 4):
                yt = ypool.tile([128, D], F32, name="yt", tag="yt")
                nc.sync.dma_start(out=yt, in_=y_img[lc * 128 : (lc + 1) * 128, :])
                for dc in range(N_DC):
                    pt = tr_tile()
                    nc.tensor.transpose(pt, yt[:, dc * 128 : (dc + 1) * 128], ident)
                    dst = yT[dc][:, lc * 128 : (lc + 1) * 128]
                    if flip % 2 == 0:
                        nc.vector.tensor_copy(dst, pt)
                    else:
                        nc.scalar.copy(dst, pt)
                    flip += 1

            # ep = exp(M.T @ y.T + b2) for this half
            ps_u = sm_tile(K, 512)
            for dc in range(N_DC):
                nc.tensor.matmul(ps_u, m_sb[:, dc, :], yT[dc][:, jsl],
                                 start=(dc == 0), stop=(dc == N_DC - 1))
            nc.scalar.activation(ep[:, jsl], ps_u,
                                 mybir.ActivationFunctionType.Exp,
                                 bias=ebias, scale=1.0)

            # lz chunks for this half + bf16 splits + flatten into rhs rows
            for jc in range(jh * 4, (jh + 1) * 4):
                ps_nd = sm_tile(128, C + 1)
                nc.tensor.matmul(ps_nd, ep[:, jc * 128 : (jc + 1) * 128], gb)
                recip = lz_pool.tile([128, 1], F32, name="recip", tag="recip")
                nc.vector.reciprocal(recip, ps_nd[:, C : C + 1])
                nc.vector.scalar_tensor_tensor(
                    lz_sb[:, jc, :], ps_nd[:, 0:C], recip, csb_rep,
                    op0=mybir.AluOpType.mult, op1=mybir.AluOpType.add)
            hsl = slice(jh * 4, (jh + 1) * 4)
            nc.vector.tensor_copy(lz_hi[:, hsl, :], lz_sb[:, hsl, :])
            nc.vector.tensor_sub(lz_rem1[:, hsl, :], lz_sb[:, hsl, :], lz_hi[:, hsl, :])
            nc.vector.tensor_copy(lz_mid[:, hsl, :], lz_rem1[:, hsl, :])
            nc.vector.tensor_sub(lz_rem2[:, hsl, :], lz_rem1[:, hsl, :], lz_mid[:, hsl, :])
            nc.vector.tensor_copy(lz_lo[:, hsl, :], lz_rem2[:, hsl, :])
            for s in range(3):
                for ch in range(4):
                    nc.scalar.dma_start(
                        out=rhs[jh][96 + s : 97 + s, ch * 128 * C : (ch + 1) * 128 * C],
                        in_=lz_splits[s][:, jh * 4 + ch, :],
                    )

            # outer sum for this j-half
            for ic in range(N_IC):
                lhs = ly_lhsT[:, ic * 128 : (ic + 1) * 128]
                for qg in range(Q_N // OUT_Q):
                    ob = outpool.tile([128, OUT_W], F32, name="ob", tag="ob")
                    for qq in range(OUT_Q):
                        q = qg * OUT_Q + qq
                        po = out_ps.tile([128, 512], F32, name="po", tag="po")
                        nc.tensor.matmul(po, lhs, rhs[jh][:, q * 512 : (q + 1) * 512])
                        dst = ob[:, qq * 512 : (qq + 1) * 512]
                        if q % 2 == 0:
                            nc.vector.tensor_copy(dst, po)
                        else:
                            nc.scalar.copy(dst, po)
                    nc.sync.dma_start(
                        out=out[ic * 128 : (ic + 1) * 128,
                                jh * RHS_W + qg * OUT_W : jh * RHS_W + (qg + 1) * OUT_W],
                        in_=ob,
                    )


_NC_CACHE = None


def _get_nc():
    global _NC_CACHE
    if _NC_CACHE is None:
        _NC_CACHE = _build_program()
    return _NC_CACHE


def make_in_maps(inputs):
    x = np.ascontiguousarray(np.asarray(inputs["x"], dtype=np.float32))
    small = {
        k: np.ascontiguousarray(np.asarray(inputs[k], dtype=np.float32))
        for k in ("dic", "prior", "Wy_w", "Wy_b", "Wz_w", "Wz_b", "cs_w", "cs_b")
    }
    in_maps = []
    for core in range(8):
        b, ihalf = core % B, core // B
        in_maps.append({
            "y_img": x[b],
            "y_half": np.ascontiguousarray(x[b, ihalf * HALF_L : (ihalf + 1) * HALF_L]),
            **small,
        })
    return in_maps


def assemble(results):
    out = np.empty((B, L, L, C), dtype=np.float32)
    for core in range(8):
        b, ihalf = core % B, core // B
        out[b, ihalf * HALF_L : (ihalf + 1) * HALF_L] = (
            results[core]["out_loc"].reshape(HALF_L, L, C)
        )
    return out.reshape(B, L * L, C)


def _install_trace_support():
    """The agent image's antenv lacks axon_hooks, so boot() skipped NTFF hook
    install. Recreate the module and register the ctypes-based hook; also stub
    the S3 artifact upload (no creds in this container)."""
    import types

    if sys.modules.get("antenv.axon_hooks") is None:
        mod = types.ModuleType("antenv.axon_hooks")
        _hook = [None]
        mod.set_axon_ntff_profile_hook = lambda h: _hook.__setitem__(0, h)
        mod.get_axon_ntff_profile_hook = lambda: _hook[0]
        sys.modules["antenv.axon_hooks"] = mod
        import antenv

        antenv.axon_hooks = mod
    import antenv.axon_hooks as ah

    if ah.get_axon_ntff_profile_hook() is None:
        from trn_agent_boot.trn_boot import _ntff_profile_via_ctypes

        ah.set_axon_ntff_profile_hook(
            _ntff_profile_via_ctypes("/opt/axon/libaxon_pjrt.so")
        )
    import concourse.bass_utils as bu

    bu.upload_artifacts = lambda tmpdir: tmpdir


def run(inputs, trace=False, **kw):
    from concourse.bass_utils import run_bass_kernel_spmd

    if trace:
        _install_trace_support()
    nc = _get_nc()
    res = run_bass_kernel_spmd(
        nc, make_in_maps(inputs), core_ids=list(range(8)), trace=trace, **kw
    )
    return assemble(res.results), res


def kernel(**inputs) -> np.ndarray:
    out, _ = run(inputs, trace=False)
    return out


# revision 18
# speedup vs baseline: 1.5998x; 1.0494x over previous
"""Trainium2 Bass kernel for nn_CausalPredictor.

Math (per image y = x[b], all f32):
    zd   = dic @ Wz_w.T + Wz_b                          [K, C]
    att  = softmax((y @ Wy_w.T + Wy_b) @ zd.T * s, k)   [L, K]
    z    = (att * prior) @ dic                          [L, D]
    ly   = y @ cs_w[:, :D].T                            [L, C]
    lz   = z @ cs_w[:, D:].T + cs_b                     [L, C]
    out[i*L+j, c] = ly[i, c] + lz[j, c]                 [L*L, C]

Device-side rewrite (keeps every big matmul contraction on the partition dim
and never materializes z):
    zdts = (zd.T + Wz_b) * s            [C, K]
    M    = Wy_w.T @ zdts                [D, K]
    b2   = Wy_b @ zdts                  [K]
    ep   = exp(M.T @ y.T + b2)          [K, L]   (logits, transposed)
    G    = diag(prior) @ (dic @ csz.T)  [K, C];  Gb = [G | ones]  [K, C+1]
    nd   = ep_slice.T @ Gb              [128j, C+1]  (num | denom)
    lz   = nd[:, :C] / nd[:, C:] + cs_b
    out block = lhsT.T @ rhs            (K=99 bf16 matmul per 128-row block)

The outer sum runs on the PE in bf16 at 1 cycle/row (fp32 matmul is 4):
both ly and lz are split into hi+mid+lo bf16 mantissa parts, the rhs holds
three copies of a tiled identity (rows 32s..32s+20) plus the three lz_flat
splits (rows 96..98), and the lhsT holds the three lyT splits plus ones rows
(96..98).  Every product is value * {0,1} (exact in bf16), accumulated in
fp32 PSUM, so the result is fp32-exact to ~2^-24.

Sharding: 8 cores = 4 images x 2 halves of the i dim, no collectives.  The
host hands each core its image with its OWN i-half first (y_perm), so the
first 4 row-chunks feed both the ly path and the first j-half's attention
path; the host un-permutes the j-halves when assembling.  All work runs
per-128-row chunk so output DMA starts as early as possible and the second
half's compute hides under the first half's output DMA.
"""

import sys

for _p in ("/opt/trn_rl_repo", "/root/.axon_site/_ro/trn_rl_repo"):
    if _p not in sys.path:
        sys.path.append(_p)

import numpy as np

import concourse.bass as bass
from concourse import bacc
import concourse.mybir as mybir
import concourse.tile as tile
from concourse.masks import make_identity
from contextlib import ExitStack

B, L, D, K, C = 4, 1024, 1024, 20, 21
SCALE = 1.0 / float(np.sqrt(np.float32(C)))
F32 = mybir.dt.float32
BF16 = mybir.dt.bfloat16
HALF_L = L // 2          # 512 rows of i per core
N_IC = HALF_L // 128     # 4 i-chunks of 128 per core
N_DC = D // 128          # 8 chunks along the contraction dim
JC = 512                 # j columns covered by one rhs tile (one j-half)
RHS_W = JC * C           # 10752 free elements per rhs tile
Q_N = RHS_W // 512       # 21 matmuls of N=512 per (jh, ic)
OUT_Q = 7                # q's per staged output tile
OUT_W = OUT_Q * 512      # 3584 f32 per partition per staged tile


def _build_program():
    nc = bacc.Bacc(
        "TRN2",
        target_bir_lowering=False,
        debug=False,
        enable_asserts=False,
        num_devices=8,
    )
    y_perm = nc.dram_tensor("y_perm", [L, D], F32, kind="ExternalInput").ap()
    dic = nc.dram_tensor("dic", [K, D], F32, kind="ExternalInput").ap()
    prior = nc.dram_tensor("prior", [K], F32, kind="ExternalInput").ap()
    wy_w = nc.dram_tensor("Wy_w", [C, D], F32, kind="ExternalInput").ap()
    wy_b = nc.dram_tensor("Wy_b", [C], F32, kind="ExternalInput").ap()
    wz_w = nc.dram_tensor("Wz_w", [C, D], F32, kind="ExternalInput").ap()
    wz_b = nc.dram_tensor("Wz_b", [C], F32, kind="ExternalInput").ap()
    cs_w = nc.dram_tensor("cs_w", [C, 2 * D], F32, kind="ExternalInput").ap()
    cs_b = nc.dram_tensor("cs_b", [C], F32, kind="ExternalInput").ap()
    out = nc.dram_tensor("out_loc", [HALF_L, L * C], F32, kind="ExternalOutput").ap()

    with tile.TileContext(nc) as tc:
        _emit(tc, out, y_perm, dic, prior, wy_w, wy_b, wz_w, wz_b, cs_w, cs_b)
    nc.compile()
    return nc


def _bcast_ap(ap, parts):
    """Partition-broadcast a 1-D DRAM AP across `parts` partitions (DMA only)."""
    return bass.AP(tensor=ap.tensor, offset=ap.offset, ap=[[0, parts]] + list(ap.ap))


def _emit(tc, out, y_perm, dic, prior, wy_w, wy_b, wz_w, wz_b, cs_w, cs_b):
    nc = tc.nc
    ctx = ExitStack()
    with ctx:
        consts = ctx.enter_context(tc.tile_pool(name="consts", bufs=1))
        ypool = ctx.enter_context(tc.tile_pool(name="ypool", bufs=3))
        outpool = ctx.enter_context(tc.tile_pool(name="outpool", bufs=3))
        small = ctx.enter_context(tc.tile_pool(name="small", bufs=2))
        # One flat PSUM layout, no pool releases (releases serialize phases):
        # tr 2 banks + sm 2 banks + out 4 banks = 8.
        tr_ps = ctx.enter_context(tc.tile_pool(name="tr_ps", bufs=2, space="PSUM"))
        sm_ps = ctx.enter_context(tc.tile_pool(name="sm_ps", bufs=2, space="PSUM"))
        out_ps = ctx.enter_context(tc.tile_pool(name="out_ps", bufs=4, space="PSUM"))

        def tr_tile():
            return tr_ps.tile([128, 128], F32, name="tr", tag="tr")

        def sm_tile(p, f):
            return sm_ps.tile([p, f], F32, name="sm", tag="sm")

        # ---- constant loads -------------------------------------------------
        ident = consts.tile([128, 128], F32, name="ident")
        make_identity(nc, ident)

        dic_sb = consts.tile([K, D], F32, name="dic_sb")
        nc.sync.dma_start(out=dic_sb, in_=dic)
        wy_sb = consts.tile([C, D], F32, name="wy_sb")
        nc.sync.dma_start(out=wy_sb, in_=wy_w)
        wz_sb = consts.tile([C, D], F32, name="wz_sb")
        nc.sync.dma_start(out=wz_sb, in_=wz_w)
        cs_sb = consts.tile([C, 2 * D], F32, name="cs_sb")
        nc.sync.dma_start(out=cs_sb, in_=cs_w)

        prior_col = consts.tile([K, 1], F32, name="prior_col")
        nc.sync.dma_start(out=prior_col, in_=prior.unsqueeze(1))
        wyb_col = consts.tile([C, 1], F32, name="wyb_col")
        nc.sync.dma_start(out=wyb_col, in_=wy_b.unsqueeze(1))
        wzb_col = consts.tile([C, 1], F32, name="wzb_col")
        nc.sync.dma_start(out=wzb_col, in_=wz_b.unsqueeze(1))
        csb_rep = consts.tile([128, C], F32, name="csb_rep")
        nc.sync.dma_start(out=csb_rep, in_=_bcast_ap(cs_b, 128))

        # ---- prologue: transposed weights + tiny matmuls --------------------
        dicT = consts.tile([128, N_DC, K], F32, name="dicT")
        wzT = consts.tile([128, N_DC, C], F32, name="wzT")
        csyT = consts.tile([128, N_DC, C], F32, name="csyT")
        cszT = consts.tile([128, N_DC, C], F32, name="cszT")
        for dc in range(N_DC):
            sl = slice(dc * 128, (dc + 1) * 128)
            for src, dst, kk in (
                (dic_sb[:, sl], dicT[:, dc, :], K),
                (wz_sb[:, sl], wzT[:, dc, :], C),
                (cs_sb[:, sl], csyT[:, dc, :], C),
                (cs_sb[:, D + dc * 128 : D + (dc + 1) * 128], cszT[:, dc, :], C),
            ):
                pt = tr_tile()
                nc.tensor.transpose(pt[:, :kk], src, ident[:kk, :kk])
                nc.scalar.copy(dst, pt[:, :kk])

        # zdts = (Wz @ dic.T + Wz_b) * scale      [C, K]
        ps_zd = sm_tile(C, K)
        for dc in range(N_DC):
            nc.tensor.matmul(ps_zd, wzT[:, dc, :], dicT[:, dc, :],
                             start=(dc == 0), stop=(dc == N_DC - 1))
        zdts = consts.tile([C, K], F32, name="zdts")
        nc.vector.tensor_scalar(zdts, ps_zd, wzb_col, SCALE,
                                op0=mybir.AluOpType.add, op1=mybir.AluOpType.mult)

        # M = Wy_w.T @ zdts   [D, K] in 8 chunks of [128, K]
        m_sb = consts.tile([128, N_DC, K], F32, name="m_sb")
        for dc in range(N_DC):
            ps_m = sm_tile(128, K)
            nc.tensor.matmul(ps_m, wy_sb[:, dc * 128 : (dc + 1) * 128], zdts)
            nc.scalar.copy(m_sb[:, dc, :], ps_m)

        # b2 = Wy_b @ zdts -> column [K, 1] (exp bias)
        ps_b2 = sm_tile(1, K)
        nc.tensor.matmul(ps_b2, wyb_col, zdts)
        b2_row = consts.tile([1, K], F32, name="b2_row")
        nc.scalar.copy(b2_row, ps_b2)
        ps_b2t = sm_tile(K, 1)
        nc.tensor.transpose(ps_b2t, b2_row, ident[:1, :1])
        ebias = consts.tile([K, 1], F32, name="ebias")
        nc.scalar.copy(ebias, ps_b2t)

        # Gb = [diag(prior) @ dic @ csz.T | ones]   [K, C+1]
        ps_g = sm_tile(K, C)
        for dc in range(N_DC):
            nc.tensor.matmul(ps_g, dicT[:, dc, :], cszT[:, dc, :],
                             start=(dc == 0), stop=(dc == N_DC - 1))
        gb = consts.tile([K, C + 1], F32, name="gb")
        nc.vector.tensor_scalar_mul(gb[:, 0:C], ps_g, prior_col)
        nc.vector.memset(gb[:, C : C + 1], 1.0)

        # rhs tiles (one per processed j-half): rows 32s..32s+20 = tiled I_C
        # per split, rows 96..98 = bf16 splits of this half's lz_flat.
        rhs = [consts.tile([99, RHS_W], BF16, name=f"rhs{h}") for h in range(2)]
        nc.gpsimd.memset(rhs[0], 0.0)
        nc.gpsimd.memset(rhs[1], 0.0)
        nc.gpsimd.affine_select(
            out=rhs[0][0:C, :].rearrange("p (j c) -> p j c", c=C),
            in_=rhs[0][0:C, :].rearrange("p (j c) -> p j c", c=C),
            compare_op=mybir.AluOpType.not_equal,
            fill=1.0,
            base=0,
            pattern=[[0, JC], [1, C]],
            channel_multiplier=-1,
        )
        nc.gpsimd.dma_start(out=rhs[0][32 : 32 + C, :], in_=rhs[0][0:C, :])
        nc.gpsimd.dma_start(out=rhs[0][64 : 64 + C, :], in_=rhs[0][0:C, :])

        # ly lhsT skeleton (rows filled per chunk below)
        ly_lhsT = consts.tile([99, HALF_L], BF16, name="ly_lhsT")
        nc.vector.memset(ly_lhsT, 0.0)
        nc.vector.memset(ly_lhsT[96:99, :], 1.0)

        yT = [consts.tile([128, L], F32, name=f"yT{dc}") for dc in range(N_DC)]
        ep = consts.tile([K, L], F32, name="ep")
        lz_sb = consts.tile([128, N_DC, C], F32, name="lz_sb")
        lz_hi = consts.tile([128, N_DC, C], BF16, name="lz_hi")
        lz_mid = consts.tile([128, N_DC, C], BF16, name="lz_mid")
        lz_lo = consts.tile([128, N_DC, C], BF16, name="lz_lo")
        lz_rem1 = consts.tile([128, N_DC, C], F32, name="lz_rem1")
        lz_rem2 = consts.tile([128, N_DC, C], F32, name="lz_rem2")
        lz_splits = [lz_hi, lz_mid, lz_lo]
        flip = 0

        for h in range(2):
            if h == 1:
                # second rhs tile's identity rows (off the critical path)
                nc.gpsimd.dma_start(out=rhs[1][0:C, :], in_=rhs[0][0:C, :])
                nc.gpsimd.dma_start(out=rhs[1][32 : 32 + C, :], in_=rhs[0][0:C, :])
                nc.gpsimd.dma_start(out=rhs[1][64 : 64 + C, :], in_=rhs[0][0:C, :])
            for lc4 in range(4):
                lc = h * 4 + lc4
                csl = slice(lc * 128, (lc + 1) * 128)
                yt = ypool.tile([128, D], F32, name="yt", tag="yt")
                nc.sync.dma_start(out=yt, in_=y_perm[csl, :])
                for dc in range(N_DC):
                    pt = tr_tile()
                    nc.tensor.transpose(pt, yt[:, dc * 128 : (dc + 1) * 128], ident)
                    dst = yT[dc][:, csl]
                    if flip % 2 == 0:
                        nc.vector.tensor_copy(dst, pt)
                    else:
                        nc.scalar.copy(dst, pt)
                    flip += 1

                # ep chunk = exp(M.T @ yT_chunk + b2)
                ps_u = sm_tile(K, 128)
                for dc in range(N_DC):
                    nc.tensor.matmul(ps_u, m_sb[:, dc, :], yT[dc][:, csl],
                                     start=(dc == 0), stop=(dc == N_DC - 1))
                nc.scalar.activation(ep[:, csl], ps_u,
                                     mybir.ActivationFunctionType.Exp,
                                     bias=ebias, scale=1.0)

                # lz chunk + bf16 splits + flatten into rhs rows 96..98
                ps_nd = sm_tile(128, C + 1)
                nc.tensor.matmul(ps_nd, ep[:, csl], gb)
                recip = small.tile([128, 1], F32, name="recip", tag="recip")
                nc.vector.reciprocal(recip, ps_nd[:, C : C + 1])
                nc.vector.scalar_tensor_tensor(
                    lz_sb[:, lc, :], ps_nd[:, 0:C], recip, csb_rep,
                    op0=mybir.AluOpType.mult, op1=mybir.AluOpType.add)
                nc.vector.tensor_copy(lz_hi[:, lc, :], lz_sb[:, lc, :])
                nc.vector.tensor_sub(lz_rem1[:, lc, :], lz_sb[:, lc, :],
                                     lz_hi[:, lc, :])
                nc.vector.tensor_copy(lz_mid[:, lc, :], lz_rem1[:, lc, :])
                nc.vector.tensor_sub(lz_rem2[:, lc, :], lz_rem1[:, lc, :],
                                     lz_mid[:, lc, :])
                nc.vector.tensor_copy(lz_lo[:, lc, :], lz_rem2[:, lc, :])
                for s in range(3):
                    nc.gpsimd.dma_start(
                        out=rhs[h][96 + s : 97 + s, lc4 * 128 * C : (lc4 + 1) * 128 * C],
                        in_=lz_splits[s][:, lc, :],
                    )

                if h == 0:
                    # ly chunk: lyT cols ic*128.. -> bf16 splits into ly_lhsT
                    ps_lyc = sm_tile(C, 128)
                    for dc in range(N_DC):
                        nc.tensor.matmul(ps_lyc, csyT[:, dc, :], yT[dc][:, csl],
                                         start=(dc == 0), stop=(dc == N_DC - 1))
                    hi_b = small.tile([C, 128], BF16, name="hi_b", tag="hi_b")
                    mid_b = small.tile([C, 128], BF16, name="mid_b", tag="mid_b")
                    lo_b = small.tile([C, 128], BF16, name="lo_b", tag="lo_b")
                    rem1 = small.tile([C, 128], F32, name="rem1", tag="rem1")
                    rem2 = small.tile([C, 128], F32, name="rem2", tag="rem2")
                    nc.scalar.copy(hi_b, ps_lyc)
                    nc.vector.tensor_sub(rem1, ps_lyc, hi_b)
                    nc.scalar.copy(mid_b, rem1)
                    nc.vector.tensor_sub(rem2, rem1, mid_b)
                    nc.scalar.copy(lo_b, rem2)
                    nc.gpsimd.dma_start(out=ly_lhsT[0:C, csl], in_=hi_b)
                    nc.gpsimd.dma_start(out=ly_lhsT[32 : 32 + C, csl], in_=mid_b)
                    nc.gpsimd.dma_start(out=ly_lhsT[64 : 64 + C, csl], in_=lo_b)

            # outer sum for this processed half
            for ic in range(N_IC):
                lhs = ly_lhsT[:, ic * 128 : (ic + 1) * 128]
                for qg in range(Q_N // OUT_Q):
                    ob = outpool.tile([128, OUT_W], F32, name="ob", tag="ob")
                    for qq in range(OUT_Q):
                        q = qg * OUT_Q + qq
                        po = out_ps.tile([128, 512], F32, name="po", tag="po")
                        nc.tensor.matmul(po, lhs, rhs[h][:, q * 512 : (q + 1) * 512])
                        dst = ob[:, qq * 512 : (qq + 1) * 512]
                        if q % 2 == 0:
                            nc.vector.tensor_copy(dst, po)
                        else:
                            nc.scalar.copy(dst, po)
                    nc.sync.dma_start(
                        out=out[ic * 128 : (ic + 1) * 128,
                                h * RHS_W + qg * OUT_W : h * RHS_W + (qg + 1) * OUT_W],
                        in_=ob,
                    )


_NC_CACHE = None


def _get_nc():
    global _NC_CACHE
    if _NC_CACHE is None:
        _NC_CACHE = _build_program()
    return _NC_CACHE


def make_in_maps(inputs):
    x = np.ascontiguousarray(np.asarray(inputs["x"], dtype=np.float32))
    small = {
        k: np.ascontiguousarray(np.asarray(inputs[k], dtype=np.float32))
        for k in ("dic", "prior", "Wy_w", "Wy_b", "Wz_w", "Wz_b", "cs_w", "cs_b")
    }
    in_maps = []
    for core in range(8):
        b, ihalf = core % B, core // B
        if ihalf == 0:
            y_perm = x[b]
        else:
            y_perm = np.ascontiguousarray(
                np.concatenate([x[b, HALF_L:], x[b, :HALF_L]], axis=0)
            )
        in_maps.append({"y_perm": y_perm, **small})
    return in_maps


def assemble(results):
    out = np.empty((B, L, L, C), dtype=np.float32)
    for core in range(8):
        b, ihalf = core % B, core // B
        # device output: [512 i_local, 2 processed-half, 512 j_local, C];
        # processed half 0 covers real j-half `ihalf`, half 1 the other.
        r = results[core]["out_loc"].reshape(HALF_L, 2, JC, C)
        dst = out[b, ihalf * HALF_L : (ihalf + 1) * HALF_L]
        dst[:, ihalf * JC : (ihalf + 1) * JC] = r[:, 0]
        dst[:, (1 - ihalf) * JC : (2 - ihalf) * JC] = r[:, 1]
    return out.reshape(B, L * L, C)


def _install_trace_support():
    """The agent image's antenv lacks axon_hooks, so boot() skipped NTFF hook
    install. Recreate the module and register the ctypes-based hook; also stub
    the S3 artifact upload (no creds in this container)."""
    import types

    if sys.modules.get("antenv.axon_hooks") is None:
        mod = types.ModuleType("antenv.axon_hooks")
        _hook = [None]
        mod.set_axon_ntff_profile_hook = lambda h: _hook.__setitem__(0, h)
        mod.get_axon_ntff_profile_hook = lambda: _hook[0]
        sys.modules["antenv.axon_hooks"] = mod
        import antenv

        antenv.axon_hooks = mod
    import antenv.axon_hooks as ah

    if ah.get_axon_ntff_profile_hook() is None:
        from trn_agent_boot.trn_boot import _ntff_profile_via_ctypes

        ah.set_axon_ntff_profile_hook(
            _ntff_profile_via_ctypes("/opt/axon/libaxon_pjrt.so")
        )
    import concourse.bass_utils as bu

    bu.upload_artifacts = lambda tmpdir: tmpdir


def run(inputs, trace=False, **kw):
    from concourse.bass_utils import run_bass_kernel_spmd

    if trace:
        _install_trace_support()
    nc = _get_nc()
    res = run_bass_kernel_spmd(
        nc, make_in_maps(inputs), core_ids=list(range(8)), trace=trace, **kw
    )
    return assemble(res.results), res


def kernel(**inputs) -> np.ndarray:
    out, _ = run(inputs, trace=False)
    return out


# revision 21
# speedup vs baseline: 1.7404x; 1.0879x over previous
"""Trainium2 Bass kernel for nn_CausalPredictor.

Math (per image y = x[b], all f32):
    zd   = dic @ Wz_w.T + Wz_b                          [K, C]
    att  = softmax((y @ Wy_w.T + Wy_b) @ zd.T * s, k)   [L, K]
    z    = (att * prior) @ dic                          [L, D]
    ly   = y @ cs_w[:, :D].T                            [L, C]
    lz   = z @ cs_w[:, D:].T + cs_b                     [L, C]
    out[i*L+j, c] = ly[i, c] + lz[j, c]                 [L*L, C]

Device-side rewrite (keeps every big matmul contraction on the partition dim
and never materializes z):
    zdts = (zd.T + Wz_b) * s            [C, K]
    M    = Wy_w.T @ zdts                [D, K]
    b2   = Wy_b @ zdts                  [K]
    ep   = exp(M.T @ y.T + b2)          [K, L]   (logits, transposed)
    G    = diag(prior) @ (dic @ csz.T)  [K, C];  Gb = [G | ones]  [K, C+1]
    nd   = ep_slice.T @ Gb              [128j, C+1]  (num | denom)
    lz   = nd[:, :C] / nd[:, C:] + cs_b
    out block = lhsT.T @ rhs            (K=99 bf16 matmul per 128-row block)

The outer sum runs on the PE in bf16 at 1 cycle/row (fp32 matmul is 4):
both ly and lz are split into hi+mid+lo bf16 mantissa parts, the rhs holds
three copies of a tiled identity (rows 32s..32s+20) plus the three lz_flat
splits (rows 96..98), and the lhsT holds the three lyT splits plus ones rows
(96..98).  Every product is value * {0,1} (exact in bf16), accumulated in
fp32 PSUM, so the result is fp32-exact to ~2^-24.

Sharding: 8 cores = 4 images x 2 halves of the i dim, no collectives.  The
host hands each core its image with its OWN i-half first (y_perm), so the
first 4 row-chunks feed both the ly path and the first j-half's attention
path; the host un-permutes the j-halves when assembling.  All work runs
per-128-row chunk so output DMA starts as early as possible and the second
half's compute hides under the first half's output DMA.
"""

import sys

for _p in ("/opt/trn_rl_repo", "/root/.axon_site/_ro/trn_rl_repo"):
    if _p not in sys.path:
        sys.path.append(_p)

import numpy as np

import concourse.bass as bass
from concourse import bacc
import concourse.mybir as mybir
import concourse.tile as tile
from concourse.masks import make_identity
from contextlib import ExitStack

B, L, D, K, C = 4, 1024, 1024, 20, 21
SCALE = 1.0 / float(np.sqrt(np.float32(C)))
F32 = mybir.dt.float32
BF16 = mybir.dt.bfloat16
HALF_L = L // 2          # 512 rows of i per core
N_IC = HALF_L // 128     # 4 i-chunks of 128 per core
N_DC = D // 128          # 8 chunks along the contraction dim
JC = 512                 # j columns covered by one rhs tile (one j-half)
RHS_W = JC * C           # 10752 free elements per rhs tile
Q_N = RHS_W // 512       # 21 matmuls of N=512 per (jh, ic)
OUT_Q = 7                # q's per staged output tile
OUT_W = OUT_Q * 512      # 3584 f32 per partition per staged tile


def _build_program():
    nc = bacc.Bacc(
        "TRN2",
        target_bir_lowering=False,
        debug=False,
        enable_asserts=False,
        num_devices=8,
    )
    y_perm = nc.dram_tensor("y_perm", [L, D], F32, kind="ExternalInput").ap()
    dic = nc.dram_tensor("dic", [K, D], F32, kind="ExternalInput").ap()
    prior = nc.dram_tensor("prior", [K], F32, kind="ExternalInput").ap()
    wy_w = nc.dram_tensor("Wy_w", [C, D], F32, kind="ExternalInput").ap()
    wy_b = nc.dram_tensor("Wy_b", [C], F32, kind="ExternalInput").ap()
    wz_w = nc.dram_tensor("Wz_w", [C, D], F32, kind="ExternalInput").ap()
    wz_b = nc.dram_tensor("Wz_b", [C], F32, kind="ExternalInput").ap()
    cs_w = nc.dram_tensor("cs_w", [C, 2 * D], F32, kind="ExternalInput").ap()
    cs_b = nc.dram_tensor("cs_b", [C], F32, kind="ExternalInput").ap()
    out = nc.dram_tensor("out_loc", [HALF_L, L * C], F32, kind="ExternalOutput").ap()

    with tile.TileContext(nc) as tc:
        _emit(tc, out, y_perm, dic, prior, wy_w, wy_b, wz_w, wz_b, cs_w, cs_b)
    nc.compile()
    return nc


def _bcast_ap(ap, parts):
    """Partition-broadcast a 1-D DRAM AP across `parts` partitions (DMA only)."""
    return bass.AP(tensor=ap.tensor, offset=ap.offset, ap=[[0, parts]] + list(ap.ap))


def _emit(tc, out, y_perm, dic, prior, wy_w, wy_b, wz_w, wz_b, cs_w, cs_b):
    nc = tc.nc
    ctx = ExitStack()
    with ctx:
        consts = ctx.enter_context(tc.tile_pool(name="consts", bufs=1))
        ypool = ctx.enter_context(tc.tile_pool(name="ypool", bufs=3))
        outpool = ctx.enter_context(tc.tile_pool(name="outpool", bufs=3))
        small = ctx.enter_context(tc.tile_pool(name="small", bufs=2))
        # One flat PSUM layout, no pool releases (releases serialize phases):
        # tr 2 banks + sm 2 banks + out 4 banks = 8.
        tr_ps = ctx.enter_context(tc.tile_pool(name="tr_ps", bufs=2, space="PSUM"))
        sm_ps = ctx.enter_context(tc.tile_pool(name="sm_ps", bufs=2, space="PSUM"))
        out_ps = ctx.enter_context(tc.tile_pool(name="out_ps", bufs=4, space="PSUM"))

        def tr_tile():
            return tr_ps.tile([128, 128], F32, name="tr", tag="tr")

        def sm_tile(p, f):
            return sm_ps.tile([p, f], F32, name="sm", tag="sm")

        # ---- constant loads -------------------------------------------------
        ident = consts.tile([128, 128], F32, name="ident")
        make_identity(nc, ident)

        dic_sb = consts.tile([K, D], F32, name="dic_sb")
        nc.scalar.dma_start(out=dic_sb, in_=dic)
        wy_sb = consts.tile([C, D], F32, name="wy_sb")
        nc.scalar.dma_start(out=wy_sb, in_=wy_w)
        wz_sb = consts.tile([C, D], F32, name="wz_sb")
        nc.scalar.dma_start(out=wz_sb, in_=wz_w)
        cs_sb = consts.tile([C, 2 * D], F32, name="cs_sb")
        nc.scalar.dma_start(out=cs_sb, in_=cs_w)

        prior_col = consts.tile([K, 1], F32, name="prior_col")
        nc.scalar.dma_start(out=prior_col, in_=prior.unsqueeze(1))
        wyb_col = consts.tile([C, 1], F32, name="wyb_col")
        nc.scalar.dma_start(out=wyb_col, in_=wy_b.unsqueeze(1))
        wzb_col = consts.tile([C, 1], F32, name="wzb_col")
        nc.scalar.dma_start(out=wzb_col, in_=wz_b.unsqueeze(1))
        csb_rep = consts.tile([128, C], F32, name="csb_rep")
        nc.scalar.dma_start(out=csb_rep, in_=_bcast_ap(cs_b, 128))

        # ---- prologue: transposed weights + tiny matmuls --------------------
        dicT = consts.tile([128, N_DC, K], F32, name="dicT")
        wzT = consts.tile([128, N_DC, C], F32, name="wzT")
        csyT = consts.tile([128, N_DC, C], F32, name="csyT")
        cszT = consts.tile([128, N_DC, C], F32, name="cszT")
        for dc in range(N_DC):
            sl = slice(dc * 128, (dc + 1) * 128)
            for src, dst, kk in (
                (dic_sb[:, sl], dicT[:, dc, :], K),
                (wz_sb[:, sl], wzT[:, dc, :], C),
                (cs_sb[:, sl], csyT[:, dc, :], C),
                (cs_sb[:, D + dc * 128 : D + (dc + 1) * 128], cszT[:, dc, :], C),
            ):
                pt = tr_tile()
                nc.tensor.transpose(pt[:, :kk], src, ident[:kk, :kk])
                nc.scalar.copy(dst, pt[:, :kk])

        # zdts = (Wz @ dic.T + Wz_b) * scale      [C, K]
        ps_zd = sm_tile(C, K)
        for dc in range(N_DC):
            nc.tensor.matmul(ps_zd, wzT[:, dc, :], dicT[:, dc, :],
                             start=(dc == 0), stop=(dc == N_DC - 1))
        zdts = consts.tile([C, K], F32, name="zdts")
        nc.vector.tensor_scalar(zdts, ps_zd, wzb_col, SCALE,
                                op0=mybir.AluOpType.add, op1=mybir.AluOpType.mult)

        # M = Wy_w.T @ zdts   [D, K] in 8 chunks of [128, K]
        m_sb = consts.tile([128, N_DC, K], F32, name="m_sb")
        for dc in range(N_DC):
            ps_m = sm_tile(128, K)
            nc.tensor.matmul(ps_m, wy_sb[:, dc * 128 : (dc + 1) * 128], zdts)
            nc.scalar.copy(m_sb[:, dc, :], ps_m)

        # b2 = Wy_b @ zdts -> column [K, 1] (exp bias)
        ps_b2 = sm_tile(1, K)
        nc.tensor.matmul(ps_b2, wyb_col, zdts)
        b2_row = consts.tile([1, K], F32, name="b2_row")
        nc.scalar.copy(b2_row, ps_b2)
        ps_b2t = sm_tile(K, 1)
        nc.tensor.transpose(ps_b2t, b2_row, ident[:1, :1])
        ebias = consts.tile([K, 1], F32, name="ebias")
        nc.scalar.copy(ebias, ps_b2t)

        # Gb = [diag(prior) @ dic @ csz.T | ones]   [K, C+1]
        ps_g = sm_tile(K, C)
        for dc in range(N_DC):
            nc.tensor.matmul(ps_g, dicT[:, dc, :], cszT[:, dc, :],
                             start=(dc == 0), stop=(dc == N_DC - 1))
        gb = consts.tile([K, C + 1], F32, name="gb")
        nc.vector.tensor_scalar_mul(gb[:, 0:C], ps_g, prior_col)
        nc.vector.memset(gb[:, C : C + 1], 1.0)

        # rhs tiles (one per processed j-half): rows 32s..32s+20 = tiled I_C
        # per split, rows 96..98 = bf16 splits of this half's lz_flat.
        # Build only a [32, 2688] corner with memset+affine_select (gpsimd is
        # slow per element) and replicate the rest with SBUF->SBUF DMAs.
        rhs = [consts.tile([99, RHS_W], BF16, name=f"rhs{h}") for h in range(2)]
        quarter = RHS_W // 4  # 2688 = 128 j periods
        nc.gpsimd.memset(rhs[0][0:32, 0:quarter], 0.0)
        nc.gpsimd.affine_select(
            out=rhs[0][0:C, 0:quarter].rearrange("p (j c) -> p j c", c=C),
            in_=rhs[0][0:C, 0:quarter].rearrange("p (j c) -> p j c", c=C),
            compare_op=mybir.AluOpType.not_equal,
            fill=1.0,
            base=0,
            pattern=[[0, quarter // C], [1, C]],
            channel_multiplier=-1,
        )
        nc.gpsimd.dma_start(out=rhs[0][0:32, quarter : 2 * quarter],
                            in_=rhs[0][0:32, 0:quarter])
        nc.gpsimd.dma_start(out=rhs[0][0:32, 2 * quarter : 4 * quarter],
                            in_=rhs[0][0:32, 0 : 2 * quarter])
        nc.gpsimd.dma_start(out=rhs[0][32:64, :], in_=rhs[0][0:32, :])
        nc.gpsimd.dma_start(out=rhs[0][64:96, :], in_=rhs[0][0:32, :])

        # ly lhsT skeleton (rows filled per chunk below)
        ly_lhsT = consts.tile([99, HALF_L], BF16, name="ly_lhsT")
        nc.vector.memset(ly_lhsT, 0.0)
        nc.vector.memset(ly_lhsT[96:99, :], 1.0)

        yT = [consts.tile([128, L], F32, name=f"yT{dc}") for dc in range(N_DC)]
        ep = consts.tile([K, L], F32, name="ep")
        lz_sb = consts.tile([128, N_DC, C], F32, name="lz_sb")
        lz_hi = consts.tile([128, N_DC, C], BF16, name="lz_hi")
        lz_mid = consts.tile([128, N_DC, C], BF16, name="lz_mid")
        lz_lo = consts.tile([128, N_DC, C], BF16, name="lz_lo")
        lz_rem1 = consts.tile([128, N_DC, C], F32, name="lz_rem1")
        lz_rem2 = consts.tile([128, N_DC, C], F32, name="lz_rem2")
        lz_splits = [lz_hi, lz_mid, lz_lo]
        flip = 0

        for h in range(2):
            if h == 1:
                # second rhs tile's identity rows (off the critical path)
                nc.gpsimd.dma_start(out=rhs[1][0:96, :], in_=rhs[0][0:96, :])
            for lc4 in range(4):
                lc = h * 4 + lc4
                csl = slice(lc * 128, (lc + 1) * 128)
                yt = ypool.tile([128, D], F32, name="yt", tag="yt")
                nc.scalar.dma_start(out=yt, in_=y_perm[csl, :])
                for dc in range(N_DC):
                    pt = tr_tile()
                    nc.tensor.transpose(pt, yt[:, dc * 128 : (dc + 1) * 128], ident)
                    dst = yT[dc][:, csl]
                    if flip % 2 == 0:
                        nc.vector.tensor_copy(dst, pt)
                    else:
                        nc.scalar.copy(dst, pt)
                    flip += 1

                # ep chunk = exp(M.T @ yT_chunk + b2)
                ps_u = sm_tile(K, 128)
                for dc in range(N_DC):
                    nc.tensor.matmul(ps_u, m_sb[:, dc, :], yT[dc][:, csl],
                                     start=(dc == 0), stop=(dc == N_DC - 1))
                nc.scalar.activation(ep[:, csl], ps_u,
                                     mybir.ActivationFunctionType.Exp,
                                     bias=ebias, scale=1.0)

                # lz chunk + bf16 splits + flatten into rhs rows 96..98
                ps_nd = sm_tile(128, C + 1)
                nc.tensor.matmul(ps_nd, ep[:, csl], gb)
                recip = small.tile([128, 1], F32, name="recip", tag="recip")
                nc.vector.reciprocal(recip, ps_nd[:, C : C + 1])
                nc.vector.scalar_tensor_tensor(
                    lz_sb[:, lc, :], ps_nd[:, 0:C], recip, csb_rep,
                    op0=mybir.AluOpType.mult, op1=mybir.AluOpType.add)
                nc.vector.tensor_copy(lz_hi[:, lc, :], lz_sb[:, lc, :])
                nc.vector.tensor_sub(lz_rem1[:, lc, :], lz_sb[:, lc, :],
                                     lz_hi[:, lc, :])
                nc.vector.tensor_copy(lz_mid[:, lc, :], lz_rem1[:, lc, :])
                nc.vector.tensor_sub(lz_rem2[:, lc, :], lz_rem1[:, lc, :],
                                     lz_mid[:, lc, :])
                nc.vector.tensor_copy(lz_lo[:, lc, :], lz_rem2[:, lc, :])
                for s in range(3):
                    nc.gpsimd.dma_start(
                        out=rhs[h][96 + s : 97 + s, lc4 * 128 * C : (lc4 + 1) * 128 * C],
                        in_=lz_splits[s][:, lc, :],
                    )

                if h == 0:
                    # ly chunk: lyT cols ic*128.. -> bf16 splits into ly_lhsT
                    ps_lyc = sm_tile(C, 128)
                    for dc in range(N_DC):
                        nc.tensor.matmul(ps_lyc, csyT[:, dc, :], yT[dc][:, csl],
                                         start=(dc == 0), stop=(dc == N_DC - 1))
                    hi_b = small.tile([C, 128], BF16, name="hi_b", tag="hi_b")
                    mid_b = small.tile([C, 128], BF16, name="mid_b", tag="mid_b")
                    lo_b = small.tile([C, 128], BF16, name="lo_b", tag="lo_b")
                    rem1 = small.tile([C, 128], F32, name="rem1", tag="rem1")
                    rem2 = small.tile([C, 128], F32, name="rem2", tag="rem2")
                    nc.scalar.copy(hi_b, ps_lyc)
                    nc.vector.tensor_sub(rem1, ps_lyc, hi_b)
                    nc.scalar.copy(mid_b, rem1)
                    nc.vector.tensor_sub(rem2, rem1, mid_b)
                    nc.scalar.copy(lo_b, rem2)
                    nc.gpsimd.dma_start(out=ly_lhsT[0:C, csl], in_=hi_b)
                    nc.gpsimd.dma_start(out=ly_lhsT[32 : 32 + C, csl], in_=mid_b)
                    nc.gpsimd.dma_start(out=ly_lhsT[64 : 64 + C, csl], in_=lo_b)

            # outer sum for this processed half
            for ic in range(N_IC):
                lhs = ly_lhsT[:, ic * 128 : (ic + 1) * 128]
                for qg in range(Q_N // OUT_Q):
                    ob = outpool.tile([128, OUT_W], F32, name="ob", tag="ob")
                    for qq in range(OUT_Q):
                        q = qg * OUT_Q + qq
                        po = out_ps.tile([128, 512], F32, name="po", tag="po")
                        nc.tensor.matmul(po, lhs, rhs[h][:, q * 512 : (q + 1) * 512])
                        dst = ob[:, qq * 512 : (qq + 1) * 512]
                        if q % 2 == 0:
                            nc.vector.tensor_copy(dst, po)
                        else:
                            nc.scalar.copy(dst, po)
                    nc.sync.dma_start(
                        out=out[ic * 128 : (ic + 1) * 128,
                                h * RHS_W + qg * OUT_W : h * RHS_W + (qg + 1) * OUT_W],
                        in_=ob,
                    )


_NC_CACHE = None


def _get_nc():
    global _NC_CACHE
    if _NC_CACHE is None:
        _NC_CACHE = _build_program()
    return _NC_CACHE


def make_in_maps(inputs):
    x = np.ascontiguousarray(np.asarray(inputs["x"], dtype=np.float32))
    small = {
        k: np.ascontiguousarray(np.asarray(inputs[k], dtype=np.float32))
        for k in ("dic", "prior", "Wy_w", "Wy_b", "Wz_w", "Wz_b", "cs_w", "cs_b")
    }
    in_maps = []
    for core in range(8):
        b, ihalf = core % B, core // B
        if ihalf == 0:
            y_perm = x[b]
        else:
            y_perm = np.ascontiguousarray(
                np.concatenate([x[b, HALF_L:], x[b, :HALF_L]], axis=0)
            )
        in_maps.append({"y_perm": y_perm, **small})
    return in_maps


def assemble(results):
    out = np.empty((B, L, L, C), dtype=np.float32)
    for core in range(8):
        b, ihalf = core % B, core // B
        # device output: [512 i_local, 2 processed-half, 512 j_local, C];
        # processed half 0 covers real j-half `ihalf`, half 1 the other.
        r = results[core]["out_loc"].reshape(HALF_L, 2, JC, C)
        dst = out[b, ihalf * HALF_L : (ihalf + 1) * HALF_L]
        dst[:, ihalf * JC : (ihalf + 1) * JC] = r[:, 0]
        dst[:, (1 - ihalf) * JC : (2 - ihalf) * JC] = r[:, 1]
    return out.reshape(B, L * L, C)


def _install_trace_support():
    """The agent image's antenv lacks axon_hooks, so boot() skipped NTFF hook
    install. Recreate the module and register the ctypes-based hook; also stub
    the S3 artifact upload (no creds in this container)."""
    import types

    if sys.modules.get("antenv.axon_hooks") is None:
        mod = types.ModuleType("antenv.axon_hooks")
        _hook = [None]
        mod.set_axon_ntff_profile_hook = lambda h: _hook.__setitem__(0, h)
        mod.get_axon_ntff_profile_hook = lambda: _hook[0]
        sys.modules["antenv.axon_hooks"] = mod
        import antenv

        antenv.axon_hooks = mod
    import antenv.axon_hooks as ah

    if ah.get_axon_ntff_profile_hook() is None:
        from trn_agent_boot.trn_boot import _ntff_profile_via_ctypes

        ah.set_axon_ntff_profile_hook(
            _ntff_profile_via_ctypes("/opt/axon/libaxon_pjrt.so")
        )
    import concourse.bass_utils as bu

    bu.upload_artifacts = lambda tmpdir: tmpdir


def run(inputs, trace=False, **kw):
    from concourse.bass_utils import run_bass_kernel_spmd

    if trace:
        _install_trace_support()
    nc = _get_nc()
    res = run_bass_kernel_spmd(
        nc, make_in_maps(inputs), core_ids=list(range(8)), trace=trace, **kw
    )
    return assemble(res.results), res


def kernel(**inputs) -> np.ndarray:
    out, _ = run(inputs, trace=False)
    return out


# revision 22
# speedup vs baseline: 1.7829x; 1.0244x over previous
"""Trainium2 Bass kernel for nn_CausalPredictor.

Math (per image y = x[b], all f32):
    zd   = dic @ Wz_w.T + Wz_b                          [K, C]
    att  = softmax((y @ Wy_w.T + Wy_b) @ zd.T * s, k)   [L, K]
    z    = (att * prior) @ dic                          [L, D]
    ly   = y @ cs_w[:, :D].T                            [L, C]
    lz   = z @ cs_w[:, D:].T + cs_b                     [L, C]
    out[i*L+j, c] = ly[i, c] + lz[j, c]                 [L*L, C]

Device-side rewrite (keeps every big matmul contraction on the partition dim
and never materializes z):
    zdts = (zd.T + Wz_b) * s            [C, K]
    M    = Wy_w.T @ zdts                [D, K]
    b2   = Wy_b @ zdts                  [K]
    ep   = exp(M.T @ y.T + b2)          [K, L]   (logits, transposed)
    G    = diag(prior) @ (dic @ csz.T)  [K, C];  Gb = [G | ones]  [K, C+1]
    nd   = ep_slice.T @ Gb              [128j, C+1]  (num | denom)
    lz   = nd[:, :C] / nd[:, C:] + cs_b
    out block = lhsT.T @ rhs            (K=99 bf16 matmul per 128-row block)

The outer sum runs on the PE in bf16 at 1 cycle/row (fp32 matmul is 4):
both ly and lz are split into hi+mid+lo bf16 mantissa parts, the rhs holds
three copies of a tiled identity (rows 32s..32s+20) plus the three lz_flat
splits (rows 96..98), and the lhsT holds the three lyT splits plus ones rows
(96..98).  Every product is value * {0,1} (exact in bf16), accumulated in
fp32 PSUM, so the result is fp32-exact to ~2^-24.

Sharding: 8 cores = 4 images x 2 halves of the i dim, no collectives.  The
host hands each core its image with its OWN i-half first (y_perm), so the
first 4 row-chunks feed both the ly path and the first j-half's attention
path; the host un-permutes the j-halves when assembling.  All work runs
per-128-row chunk so output DMA starts as early as possible and the second
half's compute hides under the first half's output DMA.
"""

import sys

for _p in ("/opt/trn_rl_repo", "/root/.axon_site/_ro/trn_rl_repo"):
    if _p not in sys.path:
        sys.path.append(_p)

import numpy as np

import concourse.bass as bass
from concourse import bacc
import concourse.mybir as mybir
import concourse.tile as tile
from concourse.masks import make_identity
from contextlib import ExitStack

B, L, D, K, C = 4, 1024, 1024, 20, 21
SCALE = 1.0 / float(np.sqrt(np.float32(C)))
F32 = mybir.dt.float32
BF16 = mybir.dt.bfloat16
HALF_L = L // 2          # 512 rows of i per core
N_IC = HALF_L // 128     # 4 i-chunks of 128 per core
N_DC = D // 128          # 8 chunks along the contraction dim
JC = 512                 # j columns covered by one rhs tile (one j-half)
RHS_W = JC * C           # 10752 free elements per rhs tile
Q_N = RHS_W // 512       # 21 matmuls of N=512 per (jh, ic)
OUT_Q = 7                # q's per staged output tile
OUT_W = OUT_Q * 512      # 3584 f32 per partition per staged tile


def _build_program():
    nc = bacc.Bacc(
        "TRN2",
        target_bir_lowering=False,
        debug=False,
        enable_asserts=False,
        num_devices=8,
    )
    y_perm = nc.dram_tensor("y_perm", [L, D], F32, kind="ExternalInput").ap()
    dic = nc.dram_tensor("dic", [K, D], F32, kind="ExternalInput").ap()
    prior = nc.dram_tensor("prior", [K], F32, kind="ExternalInput").ap()
    wy_w = nc.dram_tensor("Wy_w", [C, D], F32, kind="ExternalInput").ap()
    wy_b = nc.dram_tensor("Wy_b", [C], F32, kind="ExternalInput").ap()
    wz_w = nc.dram_tensor("Wz_w", [C, D], F32, kind="ExternalInput").ap()
    wz_b = nc.dram_tensor("Wz_b", [C], F32, kind="ExternalInput").ap()
    cs_w = nc.dram_tensor("cs_w", [C, 2 * D], F32, kind="ExternalInput").ap()
    cs_b = nc.dram_tensor("cs_b", [C], F32, kind="ExternalInput").ap()
    out = nc.dram_tensor("out_loc", [HALF_L, L * C], F32, kind="ExternalOutput").ap()

    with tile.TileContext(nc) as tc:
        _emit(tc, out, y_perm, dic, prior, wy_w, wy_b, wz_w, wz_b, cs_w, cs_b)
    nc.compile()
    return nc


def _bcast_ap(ap, parts):
    """Partition-broadcast a 1-D DRAM AP across `parts` partitions (DMA only)."""
    return bass.AP(tensor=ap.tensor, offset=ap.offset, ap=[[0, parts]] + list(ap.ap))


def _emit(tc, out, y_perm, dic, prior, wy_w, wy_b, wz_w, wz_b, cs_w, cs_b):
    nc = tc.nc
    ctx = ExitStack()
    with ctx:
        consts = ctx.enter_context(tc.tile_pool(name="consts", bufs=1))
        ypool = ctx.enter_context(tc.tile_pool(name="ypool", bufs=3))
        outpool = ctx.enter_context(tc.tile_pool(name="outpool", bufs=3))
        small = ctx.enter_context(tc.tile_pool(name="small", bufs=2))
        # One flat PSUM layout, no pool releases (releases serialize phases):
        # tr 2 banks + sm 2 banks + out 4 banks = 8.
        tr_ps = ctx.enter_context(tc.tile_pool(name="tr_ps", bufs=2, space="PSUM"))
        sm_ps = ctx.enter_context(tc.tile_pool(name="sm_ps", bufs=2, space="PSUM"))
        out_ps = ctx.enter_context(tc.tile_pool(name="out_ps", bufs=4, space="PSUM"))

        def tr_tile():
            return tr_ps.tile([128, 128], F32, name="tr", tag="tr")

        def sm_tile(p, f):
            return sm_ps.tile([p, f], F32, name="sm", tag="sm")

        # ---- constant loads -------------------------------------------------
        ident = consts.tile([128, 128], F32, name="ident")
        make_identity(nc, ident)

        dic_sb = consts.tile([K, D], F32, name="dic_sb")
        nc.scalar.dma_start(out=dic_sb, in_=dic)
        wy_sb = consts.tile([C, D], F32, name="wy_sb")
        nc.scalar.dma_start(out=wy_sb, in_=wy_w)
        wz_sb = consts.tile([C, D], F32, name="wz_sb")
        nc.scalar.dma_start(out=wz_sb, in_=wz_w)
        cs_sb = consts.tile([C, 2 * D], F32, name="cs_sb")
        nc.scalar.dma_start(out=cs_sb, in_=cs_w)

        prior_col = consts.tile([K, 1], F32, name="prior_col")
        nc.scalar.dma_start(out=prior_col, in_=prior.unsqueeze(1))
        wyb_col = consts.tile([C, 1], F32, name="wyb_col")
        nc.scalar.dma_start(out=wyb_col, in_=wy_b.unsqueeze(1))
        wzb_col = consts.tile([C, 1], F32, name="wzb_col")
        nc.scalar.dma_start(out=wzb_col, in_=wz_b.unsqueeze(1))
        csb_rep = consts.tile([128, C], F32, name="csb_rep")
        nc.scalar.dma_start(out=csb_rep, in_=_bcast_ap(cs_b, 128))

        # ---- prologue: transposed weights + tiny matmuls --------------------
        dicT = consts.tile([128, N_DC, K], F32, name="dicT")
        wzT = consts.tile([128, N_DC, C], F32, name="wzT")
        csyT = consts.tile([128, N_DC, C], F32, name="csyT")
        cszT = consts.tile([128, N_DC, C], F32, name="cszT")
        for dc in range(N_DC):
            sl = slice(dc * 128, (dc + 1) * 128)
            for src, dst, kk in (
                (dic_sb[:, sl], dicT[:, dc, :], K),
                (wz_sb[:, sl], wzT[:, dc, :], C),
                (cs_sb[:, sl], csyT[:, dc, :], C),
                (cs_sb[:, D + dc * 128 : D + (dc + 1) * 128], cszT[:, dc, :], C),
            ):
                pt = tr_tile()
                nc.tensor.transpose(pt[:, :kk], src, ident[:kk, :kk])
                nc.scalar.copy(dst, pt[:, :kk])

        # zdts = (Wz @ dic.T + Wz_b) * scale      [C, K]
        ps_zd = sm_tile(C, K)
        for dc in range(N_DC):
            nc.tensor.matmul(ps_zd, wzT[:, dc, :], dicT[:, dc, :],
                             start=(dc == 0), stop=(dc == N_DC - 1))
        zdts = consts.tile([C, K], F32, name="zdts")
        nc.vector.tensor_scalar(zdts, ps_zd, wzb_col, SCALE,
                                op0=mybir.AluOpType.add, op1=mybir.AluOpType.mult)

        # M = Wy_w.T @ zdts   [D, K] in 8 chunks of [128, K]
        m_sb = consts.tile([128, N_DC, K], F32, name="m_sb")
        for dc in range(N_DC):
            ps_m = sm_tile(128, K)
            nc.tensor.matmul(ps_m, wy_sb[:, dc * 128 : (dc + 1) * 128], zdts)
            nc.scalar.copy(m_sb[:, dc, :], ps_m)

        # b2 = Wy_b @ zdts -> column [K, 1] (exp bias)
        ps_b2 = sm_tile(1, K)
        nc.tensor.matmul(ps_b2, wyb_col, zdts)
        b2_row = consts.tile([1, K], F32, name="b2_row")
        nc.scalar.copy(b2_row, ps_b2)
        ps_b2t = sm_tile(K, 1)
        nc.tensor.transpose(ps_b2t, b2_row, ident[:1, :1])
        ebias = consts.tile([K, 1], F32, name="ebias")
        nc.scalar.copy(ebias, ps_b2t)

        # Gb = [diag(prior) @ dic @ csz.T | ones]   [K, C+1]
        ps_g = sm_tile(K, C)
        for dc in range(N_DC):
            nc.tensor.matmul(ps_g, dicT[:, dc, :], cszT[:, dc, :],
                             start=(dc == 0), stop=(dc == N_DC - 1))
        gb = consts.tile([K, C + 1], F32, name="gb")
        nc.vector.tensor_scalar_mul(gb[:, 0:C], ps_g, prior_col)
        nc.vector.memset(gb[:, C : C + 1], 1.0)

        # rhs tiles (one per processed j-half): rows 32s..32s+20 = tiled I_C
        # per split, rows 96..98 = bf16 splits of this half's lz_flat.
        # Build only a [32, 2688] corner with memset+affine_select (gpsimd is
        # slow per element) and replicate the rest with SBUF->SBUF DMAs.
        rhs = [consts.tile([99, RHS_W], BF16, name=f"rhs{h}") for h in range(2)]
        quarter = RHS_W // 4  # 2688 = 128 j periods
        nc.gpsimd.memset(rhs[0][0:32, 0:quarter], 0.0)
        nc.gpsimd.affine_select(
            out=rhs[0][0:C, 0:quarter].rearrange("p (j c) -> p j c", c=C),
            in_=rhs[0][0:C, 0:quarter].rearrange("p (j c) -> p j c", c=C),
            compare_op=mybir.AluOpType.not_equal,
            fill=1.0,
            base=0,
            pattern=[[0, quarter // C], [1, C]],
            channel_multiplier=-1,
        )
        nc.gpsimd.dma_start(out=rhs[0][0:32, quarter : 2 * quarter],
                            in_=rhs[0][0:32, 0:quarter])
        nc.gpsimd.dma_start(out=rhs[0][0:32, 2 * quarter : 4 * quarter],
                            in_=rhs[0][0:32, 0 : 2 * quarter])
        nc.gpsimd.dma_start(out=rhs[0][32:64, :], in_=rhs[0][0:32, :])
        nc.gpsimd.dma_start(out=rhs[0][64:96, :], in_=rhs[0][0:32, :])

        # ly lhsT skeleton (rows filled per chunk below)
        ly_lhsT = consts.tile([99, HALF_L], BF16, name="ly_lhsT")
        nc.vector.memset(ly_lhsT, 0.0)
        nc.vector.memset(ly_lhsT[96:99, :], 1.0)

        yT = [consts.tile([128, L], F32, name=f"yT{dc}") for dc in range(N_DC)]
        ep = consts.tile([K, L], F32, name="ep")
        lz_sb = consts.tile([128, N_DC, C], F32, name="lz_sb")
        lz_hi = consts.tile([128, N_DC, C], BF16, name="lz_hi")
        lz_mid = consts.tile([128, N_DC, C], BF16, name="lz_mid")
        lz_lo = consts.tile([128, N_DC, C], BF16, name="lz_lo")
        lz_rem1 = consts.tile([128, N_DC, C], F32, name="lz_rem1")
        lz_rem2 = consts.tile([128, N_DC, C], F32, name="lz_rem2")
        lz_splits = [lz_hi, lz_mid, lz_lo]
        flip = [0]

        def chunk_attention(lc, dma_engine):
            """y chunk lc: load, transpose, ep, lz + bf16 splits + flatten."""
            h, lc4 = lc // 4, lc % 4
            csl = slice(lc * 128, (lc + 1) * 128)
            yt = ypool.tile([128, D], F32, name="yt", tag="yt")
            dma_engine.dma_start(out=yt, in_=y_perm[csl, :])
            for dc in range(N_DC):
                pt = tr_tile()
                nc.tensor.transpose(pt, yt[:, dc * 128 : (dc + 1) * 128], ident)
                dst = yT[dc][:, csl]
                if flip[0] % 2 == 0:
                    nc.vector.tensor_copy(dst, pt)
                else:
                    nc.scalar.copy(dst, pt)
                flip[0] += 1

            # ep chunk = exp(M.T @ yT_chunk + b2)
            ps_u = sm_tile(K, 128)
            for dc in range(N_DC):
                nc.tensor.matmul(ps_u, m_sb[:, dc, :], yT[dc][:, csl],
                                 start=(dc == 0), stop=(dc == N_DC - 1))
            nc.scalar.activation(ep[:, csl], ps_u,
                                 mybir.ActivationFunctionType.Exp,
                                 bias=ebias, scale=1.0)

            # lz chunk + bf16 splits + flatten into rhs rows 96..98
            ps_nd = sm_tile(128, C + 1)
            nc.tensor.matmul(ps_nd, ep[:, csl], gb)
            recip = small.tile([128, 1], F32, name="recip", tag="recip")
            nc.vector.reciprocal(recip, ps_nd[:, C : C + 1])
            nc.vector.scalar_tensor_tensor(
                lz_sb[:, lc, :], ps_nd[:, 0:C], recip, csb_rep,
                op0=mybir.AluOpType.mult, op1=mybir.AluOpType.add)
            nc.vector.tensor_copy(lz_hi[:, lc, :], lz_sb[:, lc, :])
            nc.vector.tensor_sub(lz_rem1[:, lc, :], lz_sb[:, lc, :],
                                 lz_hi[:, lc, :])
            nc.vector.tensor_copy(lz_mid[:, lc, :], lz_rem1[:, lc, :])
            nc.vector.tensor_sub(lz_rem2[:, lc, :], lz_rem1[:, lc, :],
                                 lz_mid[:, lc, :])
            nc.vector.tensor_copy(lz_lo[:, lc, :], lz_rem2[:, lc, :])
            for s in range(3):
                nc.gpsimd.dma_start(
                    out=rhs[h][96 + s : 97 + s, lc4 * 128 * C : (lc4 + 1) * 128 * C],
                    in_=lz_splits[s][:, lc, :],
                )

        def chunk_ly(lc):
            """lyT cols lc*128.. -> bf16 splits into ly_lhsT (h=0 chunks only)."""
            csl = slice(lc * 128, (lc + 1) * 128)
            ps_lyc = sm_tile(C, 128)
            for dc in range(N_DC):
                nc.tensor.matmul(ps_lyc, csyT[:, dc, :], yT[dc][:, csl],
                                 start=(dc == 0), stop=(dc == N_DC - 1))
            hi_b = small.tile([C, 128], BF16, name="hi_b", tag="hi_b")
            mid_b = small.tile([C, 128], BF16, name="mid_b", tag="mid_b")
            lo_b = small.tile([C, 128], BF16, name="lo_b", tag="lo_b")
            rem1 = small.tile([C, 128], F32, name="rem1", tag="rem1")
            rem2 = small.tile([C, 128], F32, name="rem2", tag="rem2")
            nc.scalar.copy(hi_b, ps_lyc)
            nc.vector.tensor_sub(rem1, ps_lyc, hi_b)
            nc.scalar.copy(mid_b, rem1)
            nc.vector.tensor_sub(rem2, rem1, mid_b)
            nc.scalar.copy(lo_b, rem2)
            nc.gpsimd.dma_start(out=ly_lhsT[0:C, csl], in_=hi_b)
            nc.gpsimd.dma_start(out=ly_lhsT[32 : 32 + C, csl], in_=mid_b)
            nc.gpsimd.dma_start(out=ly_lhsT[64 : 64 + C, csl], in_=lo_b)

        def outer_sum(h, ic):
            lhs = ly_lhsT[:, ic * 128 : (ic + 1) * 128]
            for qg in range(Q_N // OUT_Q):
                ob = outpool.tile([128, OUT_W], F32, name="ob", tag="ob")
                for qq in range(OUT_Q):
                    q = qg * OUT_Q + qq
                    po = out_ps.tile([128, 512], F32, name="po", tag="po")
                    nc.tensor.matmul(po, lhs, rhs[h][:, q * 512 : (q + 1) * 512])
                    dst = ob[:, qq * 512 : (qq + 1) * 512]
                    if q % 2 == 0:
                        nc.vector.tensor_copy(dst, po)
                    else:
                        nc.scalar.copy(dst, po)
                nc.sync.dma_start(
                    out=out[ic * 128 : (ic + 1) * 128,
                            h * RHS_W + qg * OUT_W : h * RHS_W + (qg + 1) * OUT_W],
                    in_=ob,
                )

        # h=0 attention+ly chunks, then phase-2 h=0 interleaved with the h=1
        # chunk pipeline (so h=1's small DVE/ACT/PE ops don't queue behind a
        # full half's worth of PSUM->SBUF copies), then phase-2 h=1.
        for lc in range(4):
            chunk_attention(lc, nc.scalar)
            chunk_ly(lc)
        nc.gpsimd.dma_start(out=rhs[1][0:96, :], in_=rhs[0][0:96, :])
        for ic in range(N_IC):
            outer_sum(0, ic)
            chunk_attention(4 + ic, nc.gpsimd)
        for ic in range(N_IC):
            outer_sum(1, ic)


_NC_CACHE = None


def _get_nc():
    global _NC_CACHE
    if _NC_CACHE is None:
        _NC_CACHE = _build_program()
    return _NC_CACHE


def make_in_maps(inputs):
    x = np.ascontiguousarray(np.asarray(inputs["x"], dtype=np.float32))
    small = {
        k: np.ascontiguousarray(np.asarray(inputs[k], dtype=np.float32))
        for k in ("dic", "prior", "Wy_w", "Wy_b", "Wz_w", "Wz_b", "cs_w", "cs_b")
    }
    in_maps = []
    for core in range(8):
        b, ihalf = core % B, core // B
        if ihalf == 0:
            y_perm = x[b]
        else:
            y_perm = np.ascontiguousarray(
                np.concatenate([x[b, HALF_L:], x[b, :HALF_L]], axis=0)
            )
        in_maps.append({"y_perm": y_perm, **small})
    return in_maps


def assemble(results):
    out = np.empty((B, L, L, C), dtype=np.float32)
    for core in range(8):
        b, ihalf = core % B, core // B
        # device output: [512 i_local, 2 processed-half, 512 j_local, C];
        # processed half 0 covers real j-half `ihalf`, half 1 the other.
        r = results[core]["out_loc"].reshape(HALF_L, 2, JC, C)
        dst = out[b, ihalf * HALF_L : (ihalf + 1) * HALF_L]
        dst[:, ihalf * JC : (ihalf + 1) * JC] = r[:, 0]
        dst[:, (1 - ihalf) * JC : (2 - ihalf) * JC] = r[:, 1]
    return out.reshape(B, L * L, C)


def _install_trace_support():
    """The agent image's antenv lacks axon_hooks, so boot() skipped NTFF hook
    install. Recreate the module and register the ctypes-based hook; also stub
    the S3 artifact upload (no creds in this container)."""
    import types

    if sys.modules.get("antenv.axon_hooks") is None:
        mod = types.ModuleType("antenv.axon_hooks")
        _hook = [None]
        mod.set_axon_ntff_profile_hook = lambda h: _hook.__setitem__(0, h)
        mod.get_axon_ntff_profile_hook = lambda: _hook[0]
        sys.modules["antenv.axon_hooks"] = mod
        import antenv

        antenv.axon_hooks = mod
    import antenv.axon_hooks as ah

    if ah.get_axon_ntff_profile_hook() is None:
        from trn_agent_boot.trn_boot import _ntff_profile_via_ctypes

        ah.set_axon_ntff_profile_hook(
            _ntff_profile_via_ctypes("/opt/axon/libaxon_pjrt.so")
        )
    import concourse.bass_utils as bu

    bu.upload_artifacts = lambda tmpdir: tmpdir


def run(inputs, trace=False, **kw):
    from concourse.bass_utils import run_bass_kernel_spmd

    if trace:
        _install_trace_support()
    nc = _get_nc()
    res = run_bass_kernel_spmd(
        nc, make_in_maps(inputs), core_ids=list(range(8)), trace=trace, **kw
    )
    return assemble(res.results), res


def kernel(**inputs) -> np.ndarray:
    out, _ = run(inputs, trace=False)
    return out
